# revision 5
# baseline (speedup 1.0000x reference)
"""Trainium2 Bass kernel for nn_MultiHeadAttention_34144990003301 (v5).

Head-parallel attention (2 heads/core), bf16 q/k/v datapath.
BatchNorm1 is POSTPONED past para_linear1: the affine commutes through
the linear layer (h1 = a1*(W1@O) + c1*rowsum(W1) + b1), so raw attention
output O is quantized to fp8e4 (scaled x32), AllGathered in quarters of
the batch DURING attention, and para_linear1 runs as fp8 DoubleRow
matmuls (2 contraction tiles per MM, W1 scaled x2^14). The BN1 bias term
enters as a tiny rank-2 matmul (D) appended to the accumulation; the
per-head scale a1 is applied after W2 (leaky(a*x) = a*leaky(x), a>0).
BN1 stats are AllGathered as 6 floats/core. One AllReduce of the W2
partials, sigmoid on device.

kernel(**inputs) takes the full unsharded inputs, returns [32,1,16,64] f32.
"""

import numpy as np

BS, HEADS, FN, SL, KN, ST = 32, 16, 124, 256, 64, 4
HID = 5000
HIDP = 5120                    # zero-padded hid
EPS = 1e-5
SLOPE = 0.01
N_CORES = 8
HL = HEADS // N_CORES          # 2 local heads per core
ROWS = HL * KN                 # 128 projected rows (per-head 64, duplicated)
T = BS * HEADS                 # 512 global tokens
HSH = HIDP // N_CORES          # 640 hid cols per core (5 blocks of 128)
IC = SL // 128                 # 2 i-chunks
NM = 64                        # DoubleRow kt-pairs (128 kt tiles / 2)
NMA = 48                       # pairs resident early (w1a)
NMB = NM - NMA                 # pairs streamed late (w1b)
NQ = 4                         # AllGather quarters
QB = BS // NQ                  # 8 batches per quarter
SCALE_W = 2.0 ** 14            # W1 fp8 scale
SCALE_O = 32.0                 # attention-output fp8 scale
INV_SCALES = 1.0 / (SCALE_W * SCALE_O)
# packed f32 const columns
PC_BQ, PC_BK, PC_BV = 0, 1, 2
PC_MASK = 3                    # 2 cols
PC_B2 = 5
PC_ONES = 6                    # value 1/128
PC_SEL = 7                     # 128 cols (rows 0:2)
PC_BC1 = 135                   # 128 cols (row 0) value 1.0
PC_BNP = 263                   # 8 cols (rows 0:2)
PC_BNP1 = 271                  # 4 cols (row 0)
PC_EYE8 = 275                  # 8 cols (rows 0:8)
PC_WB = 283                    # 640 cols (rows 0:2): w1s | b1 shard
PCW = 923
# packed bf16 cols: eye128 | w2 (5*KN) | ind0 (512) | ind1 (512)
PB_W2 = 128
PB_IND = 128 + 5 * KN
PBW = PB_IND + 2 * T

_prog = None


def _build():
    import concourse.bacc as bacc
    import concourse.tile as tile
    import concourse.mybir as mybir

    f32 = mybir.dt.float32
    bf16 = mybir.dt.bfloat16
    f8 = mybir.dt.float8e4
    AF = mybir.ActivationFunctionType
    OP = mybir.AluOpType
    DR = mybir.MatmulPerfMode.DoubleRow
    RG = [list(range(N_CORES))]

    nc = bacc.Bacc("TRN2", target_bir_lowering=False, debug=False,
                   num_devices=N_CORES)

    def din(name, shape, dt=f32):
        return nc.dram_tensor(
            name, list(shape), dt, kind="ExternalInput"
        ).ap()

    q_d = din("qh", (FN, BS * SL), bf16)
    k_d = din("kh", (FN, BS * SL), bf16)
    v_d = din("vh", (FN, BS * SL), bf16)
    wq_d = din("wqT", (FN, ROWS), bf16)
    wk_d = din("wkT", (FN, ROWS), bf16)
    wv_d = din("wvT", (FN, ROWS), bf16)
    pk_d = din("packf", (128, PCW))
    pb_d = din("packb", (128, PBW), bf16)
    w1a_d = din("w1a", (NMA, 128, 2 * HSH), f8)
    w1b_d = din("w1b", (NMB, 128, 2 * HSH), f8)
    out_d = nc.dram_tensor("out", [KN, T], f32, kind="ExternalOutput").ap()

    with tile.TileContext(nc) as tc:
        with (
            tc.tile_pool(name="persist", bufs=1) as pp,
            tc.tile_pool(name="dram", bufs=1, space="DRAM") as dp,
        ):
            pk_sb = pp.tile([128, PCW], f32, tag="packf")
            pb_sb = pp.tile([128, PBW], bf16, tag="packb")
            w1a = pp.tile([128, NMA * 2 * HSH], f8, tag="w1a")
            v1 = pp.tile([128, IC * KN * T], f8, tag="v1")
            O_all = pp.tile([128, NQ * 2048], f8, tag="oall")
            ab_sb = pp.tile([128, 6], f32, tag="absb")

            bq_sb = pk_sb[:, PC_BQ:PC_BQ + 1]
            bk_sb = pk_sb[:, PC_BK:PC_BK + 1]
            bv_sb = pk_sb[:, PC_BV:PC_BV + 1]
            mask_sb = pk_sb[:, PC_MASK:PC_MASK + 2]
            b2_sb = pk_sb[0:KN, PC_B2:PC_B2 + 1]
            ones128 = pk_sb[:, PC_ONES:PC_ONES + 1]      # value 1/128
            sel_sb = pk_sb[0:HL, PC_SEL:PC_SEL + 128]
            bc1_sb = pk_sb[0:1, PC_BC1:PC_BC1 + 128]
            bnp_sb = pk_sb[0:HL, PC_BNP:PC_BNP + 8]
            bnp1_sb = pk_sb[0:1, PC_BNP1:PC_BNP1 + 4]
            eye8_sb = pk_sb[0:8, PC_EYE8:PC_EYE8 + 8]
            wb_sb = pk_sb[0:2, PC_WB:PC_WB + 5 * 128]
            eye_sb = pb_sb[:, 0:128]
            w2_sb = [pb_sb[:, PB_W2 + j * KN:PB_W2 + (j + 1) * KN]
                     for j in range(5)]
            ind_sb = [pb_sb[0:8, PB_IND + h * T:PB_IND + (h + 1) * T]
                      for h in range(HL)]

            nc.sync.dma_start(pk_sb[:], pk_d)
            nc.scalar.dma_start(pb_sb[:], pb_d)
            w1av = w1a[:].rearrange("p (m x) -> p m x", m=NMA)
            for ch in range(3):
                nc.scalar.dma_start(
                    w1av[:, 16 * ch:16 * (ch + 1), :],
                    w1a_d[16 * ch:16 * (ch + 1)].transpose([1, 0, 2]),
                )
            # prefetch natural_log_exp act table
            dummy = pp.tile([1, 1], f32, tag="dummy")
            nc.scalar.activation(dummy[:], ones128[0:1, :], AF.Ln,
                                 bias=0.0, scale=1.0)

            # attention-output layout view:
            # col = q*2048 + ic*1024 + k*16 + hl*8 + b8
            Ov = O_all[:].rearrange("p (q a k h b) -> p q a k h b",
                                    q=NQ, a=IC, k=KN, h=HL)
            # gathered-token layout: col = ic*32768 + k*512 + t,
            # t = c*64 + hl*32 + qq*8 + b8
            v1v = v1[:].rearrange("p (a k t) -> p a k t", a=IC, k=KN)

            fgls = []
            with tc.tile_pool(name="proj", bufs=1) as prp:
                qp = prp.tile([ROWS, BS * SL], bf16, tag="qp")
                kp = prp.tile([ROWS, BS * SL], bf16, tag="kp")
                vp = prp.tile([ROWS, BS * SL], bf16, tag="vp")

                # ---------- Phase A: qp/kp/vp = W[R,:] @ x + b ----------
                with (
                    tc.tile_pool(name="xin", bufs=3) as xp,
                    tc.tile_pool(name="wts", bufs=1) as wp,
                    tc.tile_pool(name="psA", bufs=3, space="PSUM") as psA,
                    tc.tile_pool(name="stat", bufs=1) as st,
                ):
                    ws = []
                    for ti, w_d in enumerate((wq_d, wk_d, wv_d)):
                        w_sb = wp.tile([FN, ROWS], bf16, tag=f"w{ti}",
                                       name=f"w{ti}")
                        nc.sync.dma_start(w_sb[:], w_d)
                        ws.append(w_sb)
                    bnsts = [
                        st.tile([ROWS, 16 * 6], f32, tag=f"bnst{ti}",
                                name=f"bnst{ti}")
                        for ti in range(3)
                    ]
                    for ti, (x_d, b_sb, dst) in enumerate(
                        ((q_d, bq_sb, qp), (k_d, bk_sb, kp),
                         (v_d, bv_sb, vp))
                    ):
                        for xc in range(4):
                            xcs = slice(xc * 2048, (xc + 1) * 2048)
                            x_sb = xp.tile([FN, 2048], bf16, tag="xch",
                                           name=f"x{ti}_{xc}")
                            nc.sync.dma_start(x_sb[:], x_d[:, xcs])
                            for n in range(4):
                                cs = slice(xc * 2048 + n * 512,
                                           xc * 2048 + (n + 1) * 512)
                                ncs = 4 * xc + n
                                ps = psA.tile([ROWS, 512], f32, tag="proj",
                                              name=f"proj{ti}_{ncs}")
                                nc.tensor.matmul(
                                    ps[:], ws[ti][:],
                                    x_sb[:, n * 512:(n + 1) * 512])
                                nc.scalar.activation(
                                    dst[:, cs], ps[:], AF.Identity,
                                    bias=b_sb, scale=1.0,
                                )
                                nc.vector.bn_stats(
                                    bnsts[ti][:, 6 * ncs:6 * (ncs + 1)],
                                    dst[:, cs],
                                )

                    # ---------- Phase B: per-head BN affine for q/k/v ----
                    with tc.tile_pool(name="psB", bufs=1,
                                      space="PSUM") as psB:
                        AB = st.tile([HL, 6], f32, tag="AB")
                        for ti in range(3):
                            gc, bc_ = 2 * ti, 2 * ti + 1
                            mv = st.tile([ROWS, 2], f32, tag=f"mv{ti}",
                                         name=f"mv{ti}")
                            nc.vector.bn_aggr(
                                mv[:],
                                bnsts[ti][:].rearrange(
                                    "p (c s) -> p c s", s=6
                                ),
                            )
                            stat2 = st.tile([ROWS, 2], f32, tag=f"s2{ti}",
                                            name=f"s2{ti}")
                            nc.vector.tensor_copy(stat2[:, 0:1], mv[:, 0:1])
                            nc.vector.scalar_tensor_tensor(
                                stat2[:, 1:2], mv[:, 0:1], mv[:, 0:1],
                                mv[:, 1:2], op0=OP.mult, op1=OP.add,
                            )
                            hs = psB.tile([HL, 2], f32, tag=f"hs{ti}",
                                          name=f"hs{ti}")
                            nc.tensor.matmul(hs[:], mask_sb, stat2[:])
                            mean_h = st.tile([HL, 1], f32, tag=f"mh{ti}",
                                             name=f"mh{ti}")
                            nc.vector.tensor_copy(mean_h[:], hs[:, 0:1])
                            tmp = st.tile([HL, 1], f32, tag=f"tp{ti}",
                                          name=f"tp{ti}")
                            nc.vector.tensor_tensor(
                                tmp[:], mean_h[:], mean_h[:], op=OP.mult
                            )
                            var_h = st.tile([HL, 1], f32, tag=f"vh{ti}",
                                            name=f"vh{ti}")
                            nc.vector.tensor_tensor(
                                var_h[:], hs[:, 1:2], tmp[:],
                                op=OP.subtract,
                            )
                            nc.vector.tensor_scalar_add(
                                var_h[:], var_h[:], EPS
                            )
                            lnv = st.tile([HL, 1], f32, tag=f"ln{ti}",
                                          name=f"ln{ti}")
                            nc.scalar.activation(lnv[:], var_h[:], AF.Ln,
                                                 bias=0.0, scale=1.0)
                            rsq = st.tile([HL, 1], f32, tag=f"rq{ti}",
                                          name=f"rq{ti}")
                            nc.scalar.activation(rsq[:], lnv[:], AF.Exp,
                                                 bias=0.0, scale=-0.5)
                            a_h = st.tile([HL, 1], f32, tag=f"ah{ti}",
                                          name=f"ah{ti}")
                            nc.vector.tensor_tensor(
                                a_h[:], bnp_sb[:, gc:gc + 1], rsq[:],
                                op=OP.mult,
                            )
                            tmp2 = st.tile([HL, 1], f32, tag=f"t2{ti}",
                                           name=f"t2{ti}")
                            nc.vector.tensor_tensor(
                                tmp2[:], mean_h[:], a_h[:], op=OP.mult
                            )
                            nc.vector.tensor_tensor(
                                AB[:, bc_:bc_ + 1], bnp_sb[:, bc_:bc_ + 1],
                                tmp2[:], op=OP.subtract,
                            )
                            nc.vector.tensor_copy(AB[:, gc:gc + 1], a_h[:])
                        bc_ps = psB.tile([128, 6], f32, tag="bcps")
                        nc.tensor.matmul(bc_ps[:], sel_sb, AB[:])
                        nc.vector.tensor_copy(ab_sb[:], bc_ps[:])

                # ---------- Phase C: attention, 2 heads, AG quarters ----
                with (
                    tc.tile_pool(name="stage", bufs=3) as sg,
                    tc.tile_pool(name="expp", bufs=2) as epool,
                    tc.tile_pool(name="vwp", bufs=3) as vwp,
                    tc.tile_pool(name="small", bufs=4) as smp,
                    tc.tile_pool(name="ps_sc", bufs=2, space="PSUM") as pssc,
                    tc.tile_pool(name="ps_vt", bufs=2, space="PSUM") as psvt,
                    tc.tile_pool(name="ps_uo", bufs=2, space="PSUM") as psuo,
                ):
                    for b in range(BS):
                        qq, b8 = divmod(b, QB)
                        bsl = slice(b * SL, (b + 1) * SL)
                        qw2 = sg.tile([128, SL], bf16, tag="qw")
                        nc.gpsimd.tensor_scalar(
                            qw2[:], qp[:, bsl], ab_sb[:, 0:1], ab_sb[:, 1:2],
                            op0=OP.mult, op1=OP.add,
                        )
                        kw2 = sg.tile([128, SL], bf16, tag="kw")
                        nc.gpsimd.tensor_scalar(
                            kw2[:], kp[:, bsl], ab_sb[:, 2:3], ab_sb[:, 3:4],
                            op0=OP.mult, op1=OP.add,
                        )
                        vw2 = sg.tile([128, SL], bf16, tag="vw")
                        nc.vector.tensor_scalar(
                            vw2[:], vp[:, bsl], ab_sb[:, 4:5], ab_sb[:, 5:6],
                            op0=OP.mult, op1=OP.add,
                        )
                        # scores both heads: [128(j in jc), hl*512 + i]
                        sc_ps = pssc.tile([128, 1024], f32, tag="scps")
                        for hl in range(HL):
                            r = slice(KN * hl, KN * (hl + 1))
                            for jc in range(2):
                                nc.tensor.matmul(
                                    sc_ps[:, hl * 512 + jc * 256:
                                          hl * 512 + (jc + 1) * 256],
                                    kw2[r, jc * 128:(jc + 1) * 128],
                                    qw2[r, :],
                                )
                        eT = epool.tile([128, 1024], bf16, tag="expT")
                        nc.scalar.activation(
                            eT[:], sc_ps[:], AF.Exp, bias=0.0, scale=0.125,
                        )
                        # vw transposed: [128(s in jc), k both heads]
                        vt_ps = psvt.tile([128, 256], bf16, tag="vtps")
                        for jc in range(2):
                            nc.tensor.transpose(
                                vt_ps[:, jc * 128:(jc + 1) * 128],
                                vw2[:, jc * 128:(jc + 1) * 128],
                                eye_sb,
                            )
                        vws2 = vwp.tile([128, 2 * 2 * (KN + 1)], bf16,
                                        tag="vws")
                        vws2v = vws2[:].rearrange(
                            "p (a h e) -> p a h e", a=2, h=2
                        )
                        for jc in range(2):
                            nc.vector.tensor_copy(
                                vws2v[:, jc, :, 0:KN],
                                vt_ps[:, jc * 128:(jc + 1) * 128].rearrange(
                                    "p (h e) -> p h e", h=2
                                ),
                            )
                        nc.vector.memset(vws2v[:, :, :, KN:KN + 1],
                                         1.0 / SCALE_O)
                        # unnormalized o + scaled exp row sums (col KN)
                        uo = psuo.tile([128, 2 * 2 * (KN + 1)], f32,
                                       tag="uo")
                        for hl in range(HL):
                            for ic in range(IC):
                                c0 = hl * 130 + ic * 65
                                for jc in range(2):
                                    nc.tensor.matmul(
                                        uo[:, c0:c0 + KN + 1],
                                        eT[:, hl * 512 + jc * 256 + ic * 128:
                                           hl * 512 + jc * 256 +
                                           (ic + 1) * 128],
                                        vws2v[:, jc, hl, :],
                                        start=(jc == 0), stop=(jc == 1),
                                    )
                        rec = smp.tile([128, 4], f32, tag="rec")
                        nc.vector.reciprocal(
                            rec[:].rearrange("p (h i e) -> p h i e",
                                             h=2, i=2),
                            uo[:].rearrange("p (h i e) -> p h i e",
                                            h=2, i=2)[:, :, :, KN:KN + 1],
                        )
                        for hl in range(HL):
                            for ic in range(IC):
                                c0 = hl * 130 + ic * 65
                                dst = Ov[:, qq, ic, :, hl, b8]
                                rc = rec[:, 2 * hl + ic:2 * hl + ic + 1]
                                if ic == 0 and hl == 0:
                                    nc.scalar.activation(
                                        dst, uo[:, c0:c0 + KN], AF.Identity,
                                        bias=0.0, scale=rc,
                                    )
                                else:
                                    nc.vector.tensor_scalar(
                                        dst, uo[:, c0:c0 + KN], rc, None,
                                        op0=OP.mult,
                                    )
                        if b8 == QB - 1:
                            floc = dp.tile([128, 2048], f8,
                                           tag=f"floc{qq}",
                                           name=f"floc{qq}")
                            nc.sync.dma_start(
                                floc[:],
                                O_all[:, qq * 2048:(qq + 1) * 2048],
                            )
                            fgl = dp.tile([N_CORES, 128, 2048], f8,
                                          tag=f"fgl{qq}", name=f"fgl{qq}",
                                          addr_space="Shared")
                            nc.gpsimd.collective_compute(
                                "AllGather", OP.bypass, replica_groups=RG,
                                ins=[floc[:].opt()], outs=[fgl[:].opt()],
                            )
                            fgls.append(fgl)
                            for c in range(N_CORES):
                                src = fgl[c].rearrange(
                                    "p (a k h b) -> p a k h b",
                                    a=IC, k=KN, h=HL,
                                )
                                for hl in range(HL):
                                    t0 = c * 64 + hl * 32 + qq * QB
                                    nc.sync.dma_start(
                                        v1v[:, :, :, t0:t0 + QB],
                                        src[:, :, :, hl, :],
                                    )

            # ---------- Phase F: fp8 DoubleRow para_linear1 ----------
            with (
                tc.tile_pool(name="pf", bufs=1) as fp,
                tc.tile_pool(name="scrp", bufs=2) as scp,
                tc.tile_pool(name="st1", bufs=1) as st1,
                tc.tile_pool(name="psH", bufs=1, space="PSUM") as psH,
                tc.tile_pool(name="psD", bufs=2, space="PSUM") as psD,
            ):
                w1b = fp.tile([128, NMB * 2 * HSH], f8, tag="w1b")
                w1bv = w1b[:].rearrange("p (m x) -> p m x", m=NMB)
                nc.scalar.dma_start(w1bv[:], w1b_d.transpose([1, 0, 2]))
                w1am = w1a[:].rearrange("p (m t j) -> p m t j",
                                        m=NMA, t=2)
                w1bm = w1b[:].rearrange("p (m t j) -> p m t j",
                                        m=NMB, t=2)
                h1ps = [
                    psH.tile([128, T], f32, tag=f"h1_{j}", name=f"h1ps{j}")
                    for j in range(5)
                ]
                for m in range(NM):
                    lt = w1am[:, m] if m < NMA else w1bm[:, m - NMA]
                    ic, kk0 = m // 32, (m % 32) * 2
                    rhs = v1v[:, ic, kk0:kk0 + 2, :]
                    for j in range(5):
                        nc.tensor.matmul(
                            h1ps[j][:], lt[:, :, j * 128:(j + 1) * 128],
                            rhs, start=(m == 0), stop=False,
                            perf_mode=DR, skip_group_check=True,
                        )

                # ---------- BN1 stats (local heads) ----------
                Os = O_all[:].rearrange("p (x h b) -> p x h b", h=HL, b=QB)
                st2 = st1.tile([128, 4], f32, tag="st2")
                for hl in range(HL):
                    npe = NQ * IC * KN * QB      # 4096 elems/partition
                    scrap = scp.tile([128, npe], bf16, tag="scrap",
                                     name=f"scrap{hl}")
                    sum1 = st1.tile([128, 1], f32, tag=f"sum{hl}")
                    nc.vector.tensor_scalar(
                        scrap[:].rearrange("p (x b) -> p x b", b=QB),
                        Os[:, :, hl, :], 1.0, None, op0=OP.mult,
                        op1=OP.add, accum_out=sum1[:],
                    )
                    scrap2 = scp.tile([128, npe], bf16, tag="scrap",
                                      name=f"scrap2{hl}")
                    sq1 = st1.tile([128, 1], f32, tag=f"sq{hl}")
                    nc.scalar.activation(
                        scrap2[:].rearrange("p (x b) -> p x b", b=QB),
                        Os[:, :, hl, :], AF.Square, accum_out=sq1[:],
                    )
                    nc.vector.tensor_scalar_mul(
                        st2[:, 2 * hl:2 * hl + 1], sum1[:], 1.0 / npe
                    )
                    nc.vector.tensor_scalar_mul(
                        st2[:, 2 * hl + 1:2 * hl + 2], sq1[:], 1.0 / npe
                    )
                hs1 = psD.tile([1, 4], f32, tag="psd", name="hs1")
                nc.tensor.matmul(hs1[:], ones128, st2[:])
                hsb = st1.tile([1, 4], f32, tag="hsb")
                nc.vector.tensor_copy(hsb[:], hs1[:])
                # local affine params -> [A_hl0, B_hl0, A_hl1, B_hl1,
                #                         a1'_hl0, a1'_hl1]
                arst = st1.tile([1, 6], f32, tag="arst")
                for hl in range(HL):
                    ms = hsb[:, 2 * hl:2 * hl + 1]       # 32*mean
                    qs = hsb[:, 2 * hl + 1:2 * hl + 2]   # 1024*E[x^2]
                    m2 = st1.tile([1, 1], f32, tag=f"m2_{hl}")
                    nc.vector.tensor_tensor(m2[:], ms, ms, op=OP.mult)
                    v32 = st1.tile([1, 1], f32, tag=f"v32_{hl}")
                    nc.vector.tensor_tensor(v32[:], qs, m2[:],
                                            op=OP.subtract)
                    varp = st1.tile([1, 1], f32, tag=f"vp_{hl}")
                    nc.vector.tensor_scalar(
                        varp[:], v32[:], 1.0 / (SCALE_O * SCALE_O), EPS,
                        op0=OP.mult, op1=OP.add,
                    )
                    lnv = st1.tile([1, 1], f32, tag=f"lnv_{hl}")
                    nc.scalar.activation(lnv[:], varp[:], AF.Ln,
                                         bias=0.0, scale=1.0)
                    rs = st1.tile([1, 1], f32, tag=f"rs_{hl}")
                    nc.scalar.activation(rs[:], lnv[:], AF.Exp,
                                         bias=0.0, scale=-0.5)
                    a1 = st1.tile([1, 1], f32, tag=f"a1_{hl}")
                    nc.vector.tensor_tensor(
                        a1[:], bnp1_sb[:, 2 * hl:2 * hl + 1], rs[:],
                        op=OP.mult,
                    )
                    inva = st1.tile([1, 1], f32, tag=f"ia_{hl}")
                    nc.vector.reciprocal(inva[:], a1[:])
                    mm = st1.tile([1, 1], f32, tag=f"mm_{hl}")
                    nc.vector.tensor_scalar_mul(mm[:], ms, 1.0 / SCALE_O)
                    am = st1.tile([1, 1], f32, tag=f"am_{hl}")
                    nc.vector.tensor_tensor(am[:], a1[:], mm[:],
                                            op=OP.mult)
                    c1 = st1.tile([1, 1], f32, tag=f"c1_{hl}")
                    nc.vector.tensor_tensor(
                        c1[:], bnp1_sb[:, 2 * hl + 1:2 * hl + 2], am[:],
                        op=OP.subtract,
                    )
                    ci = st1.tile([1, 1], f32, tag=f"ci_{hl}")
                    nc.vector.tensor_tensor(ci[:], c1[:], inva[:],
                                            op=OP.mult)
                    nc.vector.tensor_scalar_mul(
                        arst[:, 2 * hl:2 * hl + 1], ci[:],
                        SCALE_W * SCALE_O,
                    )
                    nc.vector.tensor_scalar_mul(
                        arst[:, 2 * hl + 1:2 * hl + 2], inva[:],
                        SCALE_W * SCALE_O,
                    )
                    nc.vector.tensor_scalar_mul(
                        arst[:, 4 + hl:5 + hl], a1[:], INV_SCALES,
                    )
                arst_d = dp.tile([1, 6], f32, tag="arstd")
                nc.sync.dma_start(arst_d[:], arst[:])
                absh = dp.tile([N_CORES, 1, 6], f32, tag="absh",
                               addr_space="Shared")
                nc.gpsimd.collective_compute(
                    "AllGather", OP.bypass, replica_groups=RG,
                    ins=[arst_d[:].opt()], outs=[absh[:].opt()],
                )
                ab8 = st1.tile([N_CORES, 6], f32, tag="ab8")
                nc.sync.dma_start(ab8[:], absh[:, 0, :])
                # transpose per-head params to rows
                abT = []
                for s in range(2):
                    tp = psD.tile([2, N_CORES], f32, tag="psd",
                                  name=f"abTp{s}")
                    nc.tensor.transpose(tp[:], ab8[:, 2 * s:2 * s + 2],
                                        eye8_sb)
                    sb = st1.tile([2, N_CORES], f32, tag=f"abT{s}")
                    nc.vector.tensor_copy(sb[:], tp[:])
                    abT.append(sb)
                a1s = []
                for hl in range(HL):
                    tp = psD.tile([1, N_CORES], f32, tag="psd",
                                  name=f"a1Tp{hl}")
                    nc.tensor.transpose(tp[:], ab8[:, 4 + hl:5 + hl],
                                        eye8_sb)
                    sb = st1.tile([1, N_CORES], f32, tag=f"a1s{hl}")
                    nc.vector.tensor_copy(sb[:], tp[:])
                    a1s.append(sb)
                # D rows: dts[(j,hl)] = [8c, 128p] = A_hl[c]*w1s + B_hl[c]*b1
                dts = st1.tile([8, 10 * 128], bf16, tag="dts")
                for j in range(5):
                    for hl in range(HL):
                        dtp = psD.tile([8, 128], f32, tag="psd",
                                       name=f"dtp{j}_{hl}")
                        nc.tensor.matmul(
                            dtp[:], abT[hl][:],
                            wb_sb[:, j * 128:(j + 1) * 128],
                        )
                        nc.vector.tensor_copy(
                            dts[:, (j * 2 + hl) * 128:
                                (j * 2 + hl + 1) * 128],
                            dtp[:],
                        )
                # a1 broadcast [64, 16] (col = hl*8 + c)
                a1bc = st1.tile([KN, 16], f32, tag="a1bc")
                for hl in range(HL):
                    a1p = psD.tile([KN, N_CORES], f32, tag="psd",
                                   name=f"a1p{hl}")
                    nc.tensor.matmul(a1p[:], bc1_sb[:, 0:KN], a1s[hl][:])
                    nc.vector.tensor_copy(
                        a1bc[:, hl * 8:(hl + 1) * 8], a1p[:]
                    )
                # D-add into h1 psums
                for j in range(5):
                    for hl in range(HL):
                        nc.tensor.matmul(
                            h1ps[j][:],
                            dts[:, (j * 2 + hl) * 128:
                                (j * 2 + hl + 1) * 128],
                            ind_sb[hl],
                            start=False, stop=(hl == HL - 1),
                            skip_group_check=True,
                        )

                # ---------- leaky + W2 + scale + AllReduce + sigmoid ----
                h1sb = [
                    fp.tile([128, T], bf16, tag=f"h1s_{j}", name=f"h1s{j}")
                    for j in range(5)
                ]
                h1af = [
                    fp.tile([128, T], bf16, tag=f"h1a_{j}", name=f"h1a{j}")
                    for j in range(5)
                ]
                for j in range(5):
                    nc.scalar.activation(
                        h1af[j][:], h1ps[j][:], AF.Identity,
                        bias=0.0, scale=1.0,
                    )
                    nc.vector.scalar_tensor_tensor(
                        h1sb[j][:], h1af[j][:], SLOPE, h1af[j][:],
                        op0=OP.mult, op1=OP.max,
                    )
                ps2 = psH.tile([KN, T], f32, tag="out2")
                for j in range(5):
                    nc.tensor.matmul(
                        ps2[:], w2_sb[j], h1sb[j][:],
                        start=(j == 0), stop=(j == 4),
                    )
                o2f = fp.tile([KN, T], f32, tag="o2f")
                nc.vector.tensor_copy(o2f[:], ps2[:])
                o2s = fp.tile([KN, T], f32, tag="o2s")
                for c in range(N_CORES):
                    for hl in range(HL):
                        g = c * 64 + hl * 32
                        nc.vector.tensor_scalar(
                            o2s[:, g:g + 32], o2f[:, g:g + 32],
                            a1bc[:, hl * 8 + c:hl * 8 + c + 1], None,
                            op0=OP.mult,
                        )
                arin = dp.tile([KN, T], f32, tag="arin")
                nc.sync.dma_start(arin[:], o2s[:])
                arout = dp.tile([KN, T], f32, tag="arout",
                                addr_space="Shared")
                nc.gpsimd.collective_compute(
                    "AllReduce", OP.add, replica_groups=RG,
                    ins=[arin[:].opt()], outs=[arout[:].opt()],
                )
                arsb = fp.tile([KN, T], f32, tag="arsb")
                nc.sync.dma_start(arsb[:], arout[:])
                fin = fp.tile([KN, T], f32, tag="fin")
                nc.scalar.activation(
                    fin[:], arsb[:], AF.Sigmoid, bias=b2_sb, scale=1.0
                )
                nc.sync.dma_start(out_d, fin[:])

    nc.compile()
    return nc


def _dup_wT(W, c):
    W = np.asarray(W, np.float32)
    cols = [W[8 * c + ST * hl: 8 * c + ST * hl + KN, :].T for hl in range(HL)]
    return np.concatenate(cols, axis=1)


def _dup_b(b, c):
    b = np.asarray(b, np.float32)
    rows = [b[8 * c + ST * hl: 8 * c + ST * hl + KN] for hl in range(HL)]
    return np.ascontiguousarray(np.concatenate(rows))


def _prep_in_maps(inputs):
    import ml_dtypes

    f = np.float32
    bf = ml_dtypes.bfloat16
    f8 = ml_dtypes.float8_e4m3
    q = np.asarray(inputs["q"], f)
    k = np.asarray(inputs["k"], f)
    v = np.asarray(inputs["v"], f)
    qh = np.ascontiguousarray(
        q[:, 0].transpose(1, 0, 2).reshape(FN, BS * SL).astype(bf))
    kh = np.ascontiguousarray(
        k[:, 0].transpose(1, 0, 2).reshape(FN, BS * SL).astype(bf))
    vh = np.ascontiguousarray(
        v[:, 0].transpose(1, 0, 2).reshape(FN, BS * SL).astype(bf))
    W1 = np.asarray(inputs["W1"], f)
    W1p = np.zeros((HIDP, SL * KN), f)
    W1p[:HID] = W1
    # device contraction row ((ic*64+kk)*128+p) = orig col ((ic*128+p)*64+kk)
    W1r = W1p.reshape(HIDP, IC, 128, KN).transpose(1, 3, 2, 0).reshape(
        SL * KN, HIDP)
    # DoubleRow pairs: [m=(ic,m'), 128 p, 2 pair, HIDP]
    W1m = W1r.reshape(IC, 32, 2, 128, HIDP).transpose(0, 1, 3, 2, 4).reshape(
        NM, 128, 2, HIDP)
    w1sum = np.zeros((HIDP,), f)
    w1sum[:HID] = W1.sum(axis=1)
    b1p = np.zeros((HIDP,), f)
    b1p[:HID] = np.asarray(inputs["b1"], f)
    W2T = np.zeros((HIDP, KN), f)
    W2T[:HID] = np.asarray(inputs["W2"], f).T
    b2 = np.asarray(inputs["b2"], f)
    # leaky-relu token-head indicators: t = c*64 + hl*32 + q*8 + b8
    thead = (np.arange(T) // 32)         # c*2 + hl
    in_maps = []
    for c in range(N_CORES):
        h0 = HL * c
        packf = np.zeros((128, PCW), f)
        packf[:, PC_BQ] = _dup_b(inputs["bq"], c)
        packf[:, PC_BK] = _dup_b(inputs["bk"], c)
        packf[:, PC_BV] = _dup_b(inputs["bv"], c)
        for hl in range(HL):
            packf[KN * hl:KN * (hl + 1), PC_MASK + hl] = 1.0 / KN
            packf[hl, PC_SEL + hl * KN:PC_SEL + (hl + 1) * KN] = 1.0
        packf[0:KN, PC_B2] = b2
        packf[:, PC_ONES] = 1.0 / 128.0
        packf[0, PC_BC1:PC_BC1 + 128] = 1.0
        for hl in range(HL):
            packf[hl, PC_BNP:PC_BNP + 8] = [
                inputs["gq"][h0 + hl], inputs["beq"][h0 + hl],
                inputs["gk"][h0 + hl], inputs["bek"][h0 + hl],
                inputs["gv"][h0 + hl], inputs["bev"][h0 + hl],
                inputs["g1"][h0 + hl], inputs["be1"][h0 + hl],
            ]
        packf[0, PC_BNP1:PC_BNP1 + 4] = [
            inputs["g1"][h0], inputs["be1"][h0],
            inputs["g1"][h0 + 1], inputs["be1"][h0 + 1],
        ]
        packf[0:8, PC_EYE8:PC_EYE8 + 8] = np.eye(8, dtype=f)
        packf[0, PC_WB:PC_WB + 5 * 128] = w1sum[c * HSH:(c + 1) * HSH]
        packf[1, PC_WB:PC_WB + 5 * 128] = b1p[c * HSH:(c + 1) * HSH]
        packb = np.zeros((128, PBW), f)
        packb[:, 0:128] = np.eye(128, dtype=f)
        W2c = W2T[c * HSH:(c + 1) * HSH, :]
        for j in range(5):
            packb[:, PB_W2 + j * KN:PB_W2 + (j + 1) * KN] = \
                W2c[j * 128:(j + 1) * 128, :]
        for hl in range(HL):
            for cc in range(N_CORES):
                head = cc * 2 + hl
                packb[cc, PB_IND + hl * T:PB_IND + (hl + 1) * T] = \
                    (thead == head).astype(f)
        w1c = np.ascontiguousarray(
            (W1m[:, :, :, c * HSH:(c + 1) * HSH] * SCALE_W)
            .transpose(0, 1, 2, 3)
            .reshape(NM, 128, 2 * HSH).astype(f8))
        m = {
            "qh": qh, "kh": kh, "vh": vh,
            "wqT": np.ascontiguousarray(_dup_wT(inputs["Wq"], c).astype(bf)),
            "wkT": np.ascontiguousarray(_dup_wT(inputs["Wk"], c).astype(bf)),
            "wvT": np.ascontiguousarray(_dup_wT(inputs["Wv"], c).astype(bf)),
            "packf": packf,
            "packb": np.ascontiguousarray(packb.astype(bf)),
            "w1a": np.ascontiguousarray(w1c[:NMA]),
            "w1b": np.ascontiguousarray(w1c[NMA:]),
        }
        in_maps.append(m)
    return in_maps


def _unshard(o):
    # out cols: t = c*64 + hl*32 + q*8 + b8;  head = c*2 + hl,  b = q*8+b8
    out = (
        np.asarray(o, np.float32)
        .reshape(KN, N_CORES, HL, BS)
        .transpose(3, 1, 2, 0)
        .reshape(BS, HEADS, KN)[:, None]
    )
    return np.ascontiguousarray(out.astype(np.float32))


def kernel(**inputs):
    global _prog
    if _prog is None:
        _prog = _build()
    from concourse.bass_utils import run_bass_kernel_spmd

    in_maps = _prep_in_maps(inputs)
    res = run_bass_kernel_spmd(_prog, in_maps, list(range(N_CORES)))
    return _unshard(res.results[0]["out"])


# revision 6
# speedup vs baseline: 2.2191x; 2.2191x over previous
"""Trainium2 Bass kernel for nn_MultiHeadAttention_34144990003301 (v5).

Head-parallel attention (2 heads/core), bf16 q/k/v datapath.
BatchNorm1 is POSTPONED past para_linear1: the affine commutes through
the linear layer (h1 = a1*(W1@O) + c1*rowsum(W1) + b1), so raw attention
output O is quantized to fp8e4 (scaled x32), AllGathered in quarters of
the batch DURING attention, and para_linear1 runs as fp8 DoubleRow
matmuls (2 contraction tiles per MM, W1 scaled x2^14). The BN1 bias term
enters as a tiny rank-2 matmul (D) appended to the accumulation; the
per-head scale a1 is applied after W2 (leaky(a*x) = a*leaky(x), a>0).
BN1 stats are AllGathered as 6 floats/core. One AllReduce of the W2
partials, sigmoid on device.

kernel(**inputs) takes the full unsharded inputs, returns [32,1,16,64] f32.
"""

import numpy as np

BS, HEADS, FN, SL, KN, ST = 32, 16, 124, 256, 64, 4
HID = 5000
HIDP = 5120                    # zero-padded hid
EPS = 1e-5
SLOPE = 0.01
N_CORES = 8
HL = HEADS // N_CORES          # 2 local heads per core
ROWS = HL * KN                 # 128 projected rows (per-head 64, duplicated)
T = BS * HEADS                 # 512 global tokens
HSH = HIDP // N_CORES          # 640 hid cols per core (5 blocks of 128)
IC = SL // 128                 # 2 i-chunks
NM = 64                        # DoubleRow kt-pairs (128 kt tiles / 2)
NMA = 48                       # pairs resident early (w1a)
NMB = NM - NMA                 # pairs streamed late (w1b)
NQ = 4                         # AllGather quarters
QB = BS // NQ                  # 8 batches per quarter
SCALE_W = 2.0 ** 14            # W1 fp8 scale
SCALE_O = 32.0                 # attention-output fp8 scale
INV_SCALES = 1.0 / (SCALE_W * SCALE_O)
# packed f32 const columns
PC_BQ, PC_BK, PC_BV = 0, 1, 2
PC_MASK = 3                    # 2 cols
PC_B2 = 5
PC_ONES = 6                    # value 1/128
PC_SEL = 7                     # 128 cols (rows 0:2)
PC_BC1 = 135                   # 128 cols (row 0) value 1.0
PC_BNP = 263                   # 8 cols (rows 0:2)
PC_BNP1 = 271                  # 4 cols (row 0)
PC_EYE8 = 275                  # 8 cols (rows 0:8)
PC_WB = 283                    # 640 cols (rows 0:2): w1s | b1 shard
PCW = 923
# packed bf16 cols: eye128 | w2 (5*KN) | ind0 (512) | ind1 (512)
PB_W2 = 128
PB_IND = 128 + 5 * KN
PBW = PB_IND + 2 * T

_prog = None


def _build():
    import concourse.bacc as bacc
    import concourse.tile as tile
    import concourse.mybir as mybir

    f32 = mybir.dt.float32
    bf16 = mybir.dt.bfloat16
    f8 = mybir.dt.float8e4
    AF = mybir.ActivationFunctionType
    OP = mybir.AluOpType
    DR = mybir.MatmulPerfMode.DoubleRow
    RG = [list(range(N_CORES))]

    nc = bacc.Bacc("TRN2", target_bir_lowering=False, debug=False,
                   num_devices=N_CORES)

    def din(name, shape, dt=f32):
        return nc.dram_tensor(
            name, list(shape), dt, kind="ExternalInput"
        ).ap()

    q_d = din("qh", (FN, BS * SL), bf16)
    k_d = din("kh", (FN, BS * SL), bf16)
    v_d = din("vh", (FN, BS * SL), bf16)
    wq_d = din("wqT", (FN, ROWS), bf16)
    wk_d = din("wkT", (FN, ROWS), bf16)
    wv_d = din("wvT", (FN, ROWS), bf16)
    pk_d = din("packf", (128, PCW))
    pb_d = din("packb", (128, PBW), bf16)
    w1a_d = din("w1a", (NMA, 128, 2 * HSH), f8)
    w1b_d = din("w1b", (NMB, 128, 2 * HSH), f8)
    out_d = nc.dram_tensor("out", [KN, T], f32, kind="ExternalOutput").ap()

    with tile.TileContext(nc) as tc:
        with (
            tc.tile_pool(name="persist", bufs=1) as pp,
            tc.tile_pool(name="dram", bufs=1, space="DRAM") as dp,
        ):
            pk_sb = pp.tile([128, PCW], f32, tag="packf")
            pb_sb = pp.tile([128, PBW], bf16, tag="packb")
            w1a = pp.tile([128, NMA * 2 * HSH], f8, tag="w1a")
            v1 = pp.tile([128, IC * KN * T], f8, tag="v1")
            O_all = pp.tile([128, NQ * 2048], f8, tag="oall")
            ab_sb = pp.tile([128, 6], f32, tag="absb")

            bq_sb = pk_sb[:, PC_BQ:PC_BQ + 1]
            bk_sb = pk_sb[:, PC_BK:PC_BK + 1]
            bv_sb = pk_sb[:, PC_BV:PC_BV + 1]
            mask_sb = pk_sb[:, PC_MASK:PC_MASK + 2]
            b2_sb = pk_sb[0:KN, PC_B2:PC_B2 + 1]
            ones128 = pk_sb[:, PC_ONES:PC_ONES + 1]      # value 1/128
            sel_sb = pk_sb[0:HL, PC_SEL:PC_SEL + 128]
            bc1_sb = pk_sb[0:1, PC_BC1:PC_BC1 + 128]
            bnp_sb = pk_sb[0:HL, PC_BNP:PC_BNP + 8]
            bnp1_sb = pk_sb[0:1, PC_BNP1:PC_BNP1 + 4]
            eye8_sb = pk_sb[0:8, PC_EYE8:PC_EYE8 + 8]
            wb_sb = pk_sb[0:2, PC_WB:PC_WB + 5 * 128]
            eye_sb = pb_sb[:, 0:128]
            w2_sb = [pb_sb[:, PB_W2 + j * KN:PB_W2 + (j + 1) * KN]
                     for j in range(5)]
            ind_sb = [pb_sb[0:8, PB_IND + h * T:PB_IND + (h + 1) * T]
                      for h in range(HL)]

            nc.sync.dma_start(pk_sb[:], pk_d)
            nc.scalar.dma_start(pb_sb[:], pb_d)
            w1av = w1a[:].rearrange("p (m x) -> p m x", m=NMA)
            for ch in range(3):
                nc.scalar.dma_start(
                    w1av[:, 16 * ch:16 * (ch + 1), :],
                    w1a_d[16 * ch:16 * (ch + 1)].transpose([1, 0, 2]),
                )
            # prefetch natural_log_exp act table
            dummy = pp.tile([1, 1], f32, tag="dummy")
            nc.scalar.activation(dummy[:], ones128[0:1, :], AF.Ln,
                                 bias=0.0, scale=1.0)

            # attention-output layout view (token-major, (ic,k) inner):
            # col = q*2048 + hl*1024 + b8*128 + ic*64 + k
            Ov = O_all[:].rearrange("p (q h b a k) -> p q h b a k",
                                    q=NQ, h=HL, b=QB, a=IC)
            # gathered layout: col = t*128 + ic*64 + k,
            # t = c*64 + hl*32 + qq*8 + b8
            v1v = v1[:].rearrange("p (t a k) -> p t a k", t=T, a=IC)

            fgls = []
            with tc.tile_pool(name="proj", bufs=1) as prp:
                qp = prp.tile([ROWS, BS * SL], bf16, tag="qp")
                kp = prp.tile([ROWS, BS * SL], bf16, tag="kp")
                vp = prp.tile([ROWS, BS * SL], bf16, tag="vp")

                # ---------- Phase A: qp/kp/vp = W[R,:] @ x + b ----------
                with (
                    tc.tile_pool(name="xin", bufs=3) as xp,
                    tc.tile_pool(name="wts", bufs=1) as wp,
                    tc.tile_pool(name="psA", bufs=3, space="PSUM") as psA,
                    tc.tile_pool(name="stat", bufs=1) as st,
                ):
                    ws = []
                    for ti, w_d in enumerate((wq_d, wk_d, wv_d)):
                        w_sb = wp.tile([FN, ROWS], bf16, tag=f"w{ti}",
                                       name=f"w{ti}")
                        nc.sync.dma_start(w_sb[:], w_d)
                        ws.append(w_sb)
                    bnsts = [
                        st.tile([ROWS, 16 * 6], f32, tag=f"bnst{ti}",
                                name=f"bnst{ti}")
                        for ti in range(3)
                    ]
                    for ti, (x_d, b_sb, dst) in enumerate(
                        ((q_d, bq_sb, qp), (k_d, bk_sb, kp),
                         (v_d, bv_sb, vp))
                    ):
                        for xc in range(4):
                            xcs = slice(xc * 2048, (xc + 1) * 2048)
                            x_sb = xp.tile([FN, 2048], bf16, tag="xch",
                                           name=f"x{ti}_{xc}")
                            nc.sync.dma_start(x_sb[:], x_d[:, xcs])
                            for n in range(4):
                                cs = slice(xc * 2048 + n * 512,
                                           xc * 2048 + (n + 1) * 512)
                                ncs = 4 * xc + n
                                ps = psA.tile([ROWS, 512], f32, tag="proj",
                                              name=f"proj{ti}_{ncs}")
                                nc.tensor.matmul(
                                    ps[:], ws[ti][:],
                                    x_sb[:, n * 512:(n + 1) * 512])
                                nc.scalar.activation(
                                    dst[:, cs], ps[:], AF.Identity,
                                    bias=b_sb, scale=1.0,
                                )
                                nc.vector.bn_stats(
                                    bnsts[ti][:, 6 * ncs:6 * (ncs + 1)],
                                    dst[:, cs],
                                )

                    # ---------- Phase B: per-head BN affine for q/k/v ----
                    with tc.tile_pool(name="psB", bufs=1,
                                      space="PSUM") as psB:
                        AB = st.tile([HL, 6], f32, tag="AB")
                        for ti in range(3):
                            gc, bc_ = 2 * ti, 2 * ti + 1
                            mv = st.tile([ROWS, 2], f32, tag=f"mv{ti}",
                                         name=f"mv{ti}")
                            nc.vector.bn_aggr(
                                mv[:],
                                bnsts[ti][:].rearrange(
                                    "p (c s) -> p c s", s=6
                                ),
                            )
                            stat2 = st.tile([ROWS, 2], f32, tag=f"s2{ti}",
                                            name=f"s2{ti}")
                            nc.vector.tensor_copy(stat2[:, 0:1], mv[:, 0:1])
                            nc.vector.scalar_tensor_tensor(
                                stat2[:, 1:2], mv[:, 0:1], mv[:, 0:1],
                                mv[:, 1:2], op0=OP.mult, op1=OP.add,
                            )
                            hs = psB.tile([HL, 2], f32, tag=f"hs{ti}",
                                          name=f"hs{ti}")
                            nc.tensor.matmul(hs[:], mask_sb, stat2[:])
                            mean_h = st.tile([HL, 1], f32, tag=f"mh{ti}",
                                             name=f"mh{ti}")
                            nc.vector.tensor_copy(mean_h[:], hs[:, 0:1])
                            tmp = st.tile([HL, 1], f32, tag=f"tp{ti}",
                                          name=f"tp{ti}")
                            nc.vector.tensor_tensor(
                                tmp[:], mean_h[:], mean_h[:], op=OP.mult
                            )
                            var_h = st.tile([HL, 1], f32, tag=f"vh{ti}",
                                            name=f"vh{ti}")
                            nc.vector.tensor_tensor(
                                var_h[:], hs[:, 1:2], tmp[:],
                                op=OP.subtract,
                            )
                            nc.vector.tensor_scalar_add(
                                var_h[:], var_h[:], EPS
                            )
                            lnv = st.tile([HL, 1], f32, tag=f"ln{ti}",
                                          name=f"ln{ti}")
                            nc.scalar.activation(lnv[:], var_h[:], AF.Ln,
                                                 bias=0.0, scale=1.0)
                            rsq = st.tile([HL, 1], f32, tag=f"rq{ti}",
                                          name=f"rq{ti}")
                            nc.scalar.activation(rsq[:], lnv[:], AF.Exp,
                                                 bias=0.0, scale=-0.5)
                            a_h = st.tile([HL, 1], f32, tag=f"ah{ti}",
                                          name=f"ah{ti}")
                            nc.vector.tensor_tensor(
                                a_h[:], bnp_sb[:, gc:gc + 1], rsq[:],
                                op=OP.mult,
                            )
                            tmp2 = st.tile([HL, 1], f32, tag=f"t2{ti}",
                                           name=f"t2{ti}")
                            nc.vector.tensor_tensor(
                                tmp2[:], mean_h[:], a_h[:], op=OP.mult
                            )
                            nc.vector.tensor_tensor(
                                AB[:, bc_:bc_ + 1], bnp_sb[:, bc_:bc_ + 1],
                                tmp2[:], op=OP.subtract,
                            )
                            nc.vector.tensor_copy(AB[:, gc:gc + 1], a_h[:])
                        bc_ps = psB.tile([128, 6], f32, tag="bcps")
                        nc.tensor.matmul(bc_ps[:], sel_sb, AB[:])
                        nc.vector.tensor_copy(ab_sb[:], bc_ps[:])

                # ---------- Phase C: attention, 2 heads, AG quarters ----
                with (
                    tc.tile_pool(name="stage", bufs=3) as sg,
                    tc.tile_pool(name="expp", bufs=2) as epool,
                    tc.tile_pool(name="vwp", bufs=3) as vwp,
                    tc.tile_pool(name="small", bufs=4) as smp,
                    tc.tile_pool(name="ps_sc", bufs=2, space="PSUM") as pssc,
                    tc.tile_pool(name="ps_vt", bufs=2, space="PSUM") as psvt,
                    tc.tile_pool(name="ps_uo", bufs=2, space="PSUM") as psuo,
                ):
                    for b in range(BS):
                        qq, b8 = divmod(b, QB)
                        bsl = slice(b * SL, (b + 1) * SL)
                        qw2 = sg.tile([128, SL], bf16, tag="qw")
                        nc.gpsimd.tensor_scalar(
                            qw2[:], qp[:, bsl], ab_sb[:, 0:1], ab_sb[:, 1:2],
                            op0=OP.mult, op1=OP.add,
                        )
                        kw2 = sg.tile([128, SL], bf16, tag="kw")
                        nc.gpsimd.tensor_scalar(
                            kw2[:], kp[:, bsl], ab_sb[:, 2:3], ab_sb[:, 3:4],
                            op0=OP.mult, op1=OP.add,
                        )
                        vw2 = sg.tile([128, SL], bf16, tag="vw")
                        nc.vector.tensor_scalar(
                            vw2[:], vp[:, bsl], ab_sb[:, 4:5], ab_sb[:, 5:6],
                            op0=OP.mult, op1=OP.add,
                        )
                        # scores both heads: [128(j in jc), hl*512 + i]
                        sc_ps = pssc.tile([128, 1024], f32, tag="scps")
                        for hl in range(HL):
                            r = slice(KN * hl, KN * (hl + 1))
                            for jc in range(2):
                                nc.tensor.matmul(
                                    sc_ps[:, hl * 512 + jc * 256:
                                          hl * 512 + (jc + 1) * 256],
                                    kw2[r, jc * 128:(jc + 1) * 128],
                                    qw2[r, :],
                                )
                        eT = epool.tile([128, 1024], bf16, tag="expT")
                        nc.scalar.activation(
                            eT[:], sc_ps[:], AF.Exp, bias=0.0, scale=0.125,
                        )
                        # vw transposed: [128(s in jc), k both heads]
                        vt_ps = psvt.tile([128, 256], bf16, tag="vtps")
                        for jc in range(2):
                            nc.tensor.transpose(
                                vt_ps[:, jc * 128:(jc + 1) * 128],
                                vw2[:, jc * 128:(jc + 1) * 128],
                                eye_sb,
                            )
                        vws2 = vwp.tile([128, 2 * 2 * (KN + 1)], bf16,
                                        tag="vws")
                        vws2v = vws2[:].rearrange(
                            "p (a h e) -> p a h e", a=2, h=2
                        )
                        for jc in range(2):
                            nc.vector.tensor_copy(
                                vws2v[:, jc, :, 0:KN],
                                vt_ps[:, jc * 128:(jc + 1) * 128].rearrange(
                                    "p (h e) -> p h e", h=2
                                ),
                            )
                        nc.vector.memset(vws2v[:, :, :, KN:KN + 1],
                                         1.0 / SCALE_O)
                        # unnormalized o + scaled exp row sums (col KN)
                        uo = psuo.tile([128, 2 * 2 * (KN + 1)], f32,
                                       tag="uo")
                        for hl in range(HL):
                            for ic in range(IC):
                                c0 = hl * 130 + ic * 65
                                for jc in range(2):
                                    nc.tensor.matmul(
                                        uo[:, c0:c0 + KN + 1],
                                        eT[:, hl * 512 + jc * 256 + ic * 128:
                                           hl * 512 + jc * 256 +
                                           (ic + 1) * 128],
                                        vws2v[:, jc, hl, :],
                                        start=(jc == 0), stop=(jc == 1),
                                    )
                        rec = smp.tile([128, 4], f32, tag="rec")
                        nc.vector.reciprocal(
                            rec[:].rearrange("p (h i e) -> p h i e",
                                             h=2, i=2),
                            uo[:].rearrange("p (h i e) -> p h i e",
                                            h=2, i=2)[:, :, :, KN:KN + 1],
                        )
                        for hl in range(HL):
                            for ic in range(IC):
                                c0 = hl * 130 + ic * 65
                                dst = Ov[:, qq, hl, b8, ic, :]
                                rc = rec[:, 2 * hl + ic:2 * hl + ic + 1]
                                if ic == 0 and hl == 0:
                                    nc.scalar.activation(
                                        dst, uo[:, c0:c0 + KN], AF.Identity,
                                        bias=0.0, scale=rc,
                                    )
                                else:
                                    nc.vector.tensor_scalar(
                                        dst, uo[:, c0:c0 + KN], rc, None,
                                        op0=OP.mult,
                                    )
                        if b8 == QB - 1:
                            floc = dp.tile([128, 2048], f8,
                                           tag=f"floc{qq}",
                                           name=f"floc{qq}")
                            nc.sync.dma_start(
                                floc[:],
                                O_all[:, qq * 2048:(qq + 1) * 2048],
                            )
                            fgl = dp.tile([N_CORES, 128, 2048], f8,
                                          tag=f"fgl{qq}", name=f"fgl{qq}",
                                          addr_space="Shared")
                            nc.gpsimd.collective_compute(
                                "AllGather", OP.bypass, replica_groups=RG,
                                ins=[floc[:].opt()], outs=[fgl[:].opt()],
                            )
                            fgls.append(fgl)
                            for c in range(N_CORES):
                                for hl in range(HL):
                                    t0 = c * 64 + hl * 32 + qq * QB
                                    eng = nc.sync if c % 2 else nc.scalar
                                    eng.dma_start(
                                        v1[:, t0 * 128:(t0 + QB) * 128],
                                        fgl[c][:, hl * 1024:
                                               (hl + 1) * 1024],
                                    )

            # ---------- Phase F: fp8 DoubleRow para_linear1 ----------
            with (
                tc.tile_pool(name="pf", bufs=1) as fp,
                tc.tile_pool(name="scrp", bufs=2) as scp,
                tc.tile_pool(name="st1", bufs=1) as st1,
                tc.tile_pool(name="psH", bufs=1, space="PSUM") as psH,
                tc.tile_pool(name="psD", bufs=2, space="PSUM") as psD,
            ):
                w1b = fp.tile([128, NMB * 2 * HSH], f8, tag="w1b")
                w1bv = w1b[:].rearrange("p (m x) -> p m x", m=NMB)
                nc.scalar.dma_start(w1bv[:], w1b_d.transpose([1, 0, 2]))
                w1am = w1a[:].rearrange("p (m t j) -> p m t j",
                                        m=NMA, t=2)
                w1bm = w1b[:].rearrange("p (m t j) -> p m t j",
                                        m=NMB, t=2)
                h1ps = [
                    psH.tile([128, T], f32, tag=f"h1_{j}", name=f"h1ps{j}")
                    for j in range(5)
                ]
                for m in range(NM):
                    lt = w1am[:, m] if m < NMA else w1bm[:, m - NMA]
                    rhs = v1v[:, :, :, m].transpose([0, 2, 1])
                    for j in range(5):
                        nc.tensor.matmul(
                            h1ps[j][:], lt[:, :, j * 128:(j + 1) * 128],
                            rhs, start=(m == 0), stop=False,
                            perf_mode=DR, skip_group_check=True,
                        )

                # ---------- BN1 stats (local heads) ----------
                Os = O_all[:].rearrange("p (x h y) -> p x h y",
                                        h=HL, y=1024)
                st2 = st1.tile([128, 4], f32, tag="st2")
                for hl in range(HL):
                    npe = NQ * IC * KN * QB      # 4096 elems/partition
                    scrap = scp.tile([128, npe], bf16, tag="scrap",
                                     name=f"scrap{hl}")
                    sum1 = st1.tile([128, 1], f32, tag=f"sum{hl}")
                    nc.vector.tensor_scalar(
                        scrap[:].rearrange("p (x y) -> p x y", y=1024),
                        Os[:, :, hl, :], 1.0, None, op0=OP.mult,
                        op1=OP.add, accum_out=sum1[:],
                    )
                    scrap2 = scp.tile([128, npe], bf16, tag="scrap",
                                      name=f"scrap2{hl}")
                    sq1 = st1.tile([128, 1], f32, tag=f"sq{hl}")
                    nc.scalar.activation(
                        scrap2[:].rearrange("p (x y) -> p x y", y=1024),
                        Os[:, :, hl, :], AF.Square, accum_out=sq1[:],
                    )
                    nc.vector.tensor_scalar_mul(
                        st2[:, 2 * hl:2 * hl + 1], sum1[:], 1.0 / npe
                    )
                    nc.vector.tensor_scalar_mul(
                        st2[:, 2 * hl + 1:2 * hl + 2], sq1[:], 1.0 / npe
                    )
                hs1 = psD.tile([1, 4], f32, tag="psd", name="hs1")
                nc.tensor.matmul(hs1[:], ones128, st2[:])
                hsb = st1.tile([1, 4], f32, tag="hsb")
                nc.vector.tensor_copy(hsb[:], hs1[:])
                # local affine params -> [A_hl0, B_hl0, A_hl1, B_hl1,
                #                         a1'_hl0, a1'_hl1]
                arst = st1.tile([1, 6], f32, tag="arst")
                for hl in range(HL):
                    ms = hsb[:, 2 * hl:2 * hl + 1]       # 32*mean
                    qs = hsb[:, 2 * hl + 1:2 * hl + 2]   # 1024*E[x^2]
                    m2 = st1.tile([1, 1], f32, tag=f"m2_{hl}")
                    nc.vector.tensor_tensor(m2[:], ms, ms, op=OP.mult)
                    v32 = st1.tile([1, 1], f32, tag=f"v32_{hl}")
                    nc.vector.tensor_tensor(v32[:], qs, m2[:],
                                            op=OP.subtract)
                    varp = st1.tile([1, 1], f32, tag=f"vp_{hl}")
                    nc.vector.tensor_scalar(
                        varp[:], v32[:], 1.0 / (SCALE_O * SCALE_O), EPS,
                        op0=OP.mult, op1=OP.add,
                    )
                    lnv = st1.tile([1, 1], f32, tag=f"lnv_{hl}")
                    nc.scalar.activation(lnv[:], varp[:], AF.Ln,
                                         bias=0.0, scale=1.0)
                    rs = st1.tile([1, 1], f32, tag=f"rs_{hl}")
                    nc.scalar.activation(rs[:], lnv[:], AF.Exp,
                                         bias=0.0, scale=-0.5)
                    a1 = st1.tile([1, 1], f32, tag=f"a1_{hl}")
                    nc.vector.tensor_tensor(
                        a1[:], bnp1_sb[:, 2 * hl:2 * hl + 1], rs[:],
                        op=OP.mult,
                    )
                    inva = st1.tile([1, 1], f32, tag=f"ia_{hl}")
                    nc.vector.reciprocal(inva[:], a1[:])
                    mm = st1.tile([1, 1], f32, tag=f"mm_{hl}")
                    nc.vector.tensor_scalar_mul(mm[:], ms, 1.0 / SCALE_O)
                    am = st1.tile([1, 1], f32, tag=f"am_{hl}")
                    nc.vector.tensor_tensor(am[:], a1[:], mm[:],
                                            op=OP.mult)
                    c1 = st1.tile([1, 1], f32, tag=f"c1_{hl}")
                    nc.vector.tensor_tensor(
                        c1[:], bnp1_sb[:, 2 * hl + 1:2 * hl + 2], am[:],
                        op=OP.subtract,
                    )
                    ci = st1.tile([1, 1], f32, tag=f"ci_{hl}")
                    nc.vector.tensor_tensor(ci[:], c1[:], inva[:],
                                            op=OP.mult)
                    nc.vector.tensor_scalar_mul(
                        arst[:, 2 * hl:2 * hl + 1], ci[:],
                        SCALE_W * SCALE_O,
                    )
                    nc.vector.tensor_scalar_mul(
                        arst[:, 2 * hl + 1:2 * hl + 2], inva[:],
                        SCALE_W * SCALE_O,
                    )
                    nc.vector.tensor_scalar_mul(
                        arst[:, 4 + hl:5 + hl], a1[:], INV_SCALES,
                    )
                arst_d = dp.tile([1, 6], f32, tag="arstd")
                nc.sync.dma_start(arst_d[:], arst[:])
                absh = dp.tile([N_CORES, 1, 6], f32, tag="absh",
                               addr_space="Shared")
                nc.gpsimd.collective_compute(
                    "AllGather", OP.bypass, replica_groups=RG,
                    ins=[arst_d[:].opt()], outs=[absh[:].opt()],
                )
                ab8 = st1.tile([N_CORES, 6], f32, tag="ab8")
                nc.sync.dma_start(ab8[:], absh[:, 0, :])
                # transpose per-head params to rows
                abT = []
                for s in range(2):
                    tp = psD.tile([2, N_CORES], f32, tag="psd",
                                  name=f"abTp{s}")
                    nc.tensor.transpose(tp[:], ab8[:, 2 * s:2 * s + 2],
                                        eye8_sb)
                    sb = st1.tile([2, N_CORES], f32, tag=f"abT{s}")
                    nc.vector.tensor_copy(sb[:], tp[:])
                    abT.append(sb)
                a1s = []
                for hl in range(HL):
                    tp = psD.tile([1, N_CORES], f32, tag="psd",
                                  name=f"a1Tp{hl}")
                    nc.tensor.transpose(tp[:], ab8[:, 4 + hl:5 + hl],
                                        eye8_sb)
                    sb = st1.tile([1, N_CORES], f32, tag=f"a1s{hl}")
                    nc.vector.tensor_copy(sb[:], tp[:])
                    a1s.append(sb)
                # D rows: dts[(j,hl)] = [8c, 128p] = A_hl[c]*w1s + B_hl[c]*b1
                dts = st1.tile([8, 10 * 128], bf16, tag="dts")
                for j in range(5):
                    for hl in range(HL):
                        dtp = psD.tile([8, 128], f32, tag="psd",
                                       name=f"dtp{j}_{hl}")
                        nc.tensor.matmul(
                            dtp[:], abT[hl][:],
                            wb_sb[:, j * 128:(j + 1) * 128],
                        )
                        nc.vector.tensor_copy(
                            dts[:, (j * 2 + hl) * 128:
                                (j * 2 + hl + 1) * 128],
                            dtp[:],
                        )
                # a1 broadcast [64, 16] (col = hl*8 + c)
                a1bc = st1.tile([KN, 16], f32, tag="a1bc")
                for hl in range(HL):
                    a1p = psD.tile([KN, N_CORES], f32, tag="psd",
                                   name=f"a1p{hl}")
                    nc.tensor.matmul(a1p[:], bc1_sb[:, 0:KN], a1s[hl][:])
                    nc.vector.tensor_copy(
                        a1bc[:, hl * 8:(hl + 1) * 8], a1p[:]
                    )
                # D-add into h1 psums
                for j in range(5):
                    for hl in range(HL):
                        nc.tensor.matmul(
                            h1ps[j][:],
                            dts[:, (j * 2 + hl) * 128:
                                (j * 2 + hl + 1) * 128],
                            ind_sb[hl],
                            start=False, stop=(hl == HL - 1),
                            skip_group_check=True,
                        )

                # ---------- leaky + W2 + scale + AllReduce + sigmoid ----
                h1sb = [
                    fp.tile([128, T], bf16, tag=f"h1s_{j}", name=f"h1s{j}")
                    for j in range(5)
                ]
                h1af = [
                    fp.tile([128, T], bf16, tag=f"h1a_{j}", name=f"h1a{j}")
                    for j in range(5)
                ]
                for j in range(5):
                    nc.scalar.activation(
                        h1af[j][:], h1ps[j][:], AF.Identity,
                        bias=0.0, scale=1.0,
                    )
                    nc.vector.scalar_tensor_tensor(
                        h1sb[j][:], h1af[j][:], SLOPE, h1af[j][:],
                        op0=OP.mult, op1=OP.max,
                    )
                ps2 = psH.tile([KN, T], f32, tag="out2")
                for j in range(5):
                    nc.tensor.matmul(
                        ps2[:], w2_sb[j], h1sb[j][:],
                        start=(j == 0), stop=(j == 4),
                    )
                o2f = fp.tile([KN, T], f32, tag="o2f")
                nc.vector.tensor_copy(o2f[:], ps2[:])
                o2s = fp.tile([KN, T], f32, tag="o2s")
                for c in range(N_CORES):
                    for hl in range(HL):
                        g = c * 64 + hl * 32
                        nc.vector.tensor_scalar(
                            o2s[:, g:g + 32], o2f[:, g:g + 32],
                            a1bc[:, hl * 8 + c:hl * 8 + c + 1], None,
                            op0=OP.mult,
                        )
                arin = dp.tile([KN, T], f32, tag="arin")
                nc.sync.dma_start(arin[:], o2s[:])
                arout = dp.tile([KN, T], f32, tag="arout",
                                addr_space="Shared")
                nc.gpsimd.collective_compute(
                    "AllReduce", OP.add, replica_groups=RG,
                    ins=[arin[:].opt()], outs=[arout[:].opt()],
                )
                arsb = fp.tile([KN, T], f32, tag="arsb")
                nc.sync.dma_start(arsb[:], arout[:])
                fin = fp.tile([KN, T], f32, tag="fin")
                nc.scalar.activation(
                    fin[:], arsb[:], AF.Sigmoid, bias=b2_sb, scale=1.0
                )
                nc.sync.dma_start(out_d, fin[:])

    nc.compile()
    return nc


def _dup_wT(W, c):
    W = np.asarray(W, np.float32)
    cols = [W[8 * c + ST * hl: 8 * c + ST * hl + KN, :].T for hl in range(HL)]
    return np.concatenate(cols, axis=1)


def _dup_b(b, c):
    b = np.asarray(b, np.float32)
    rows = [b[8 * c + ST * hl: 8 * c + ST * hl + KN] for hl in range(HL)]
    return np.ascontiguousarray(np.concatenate(rows))


def _prep_in_maps(inputs):
    import ml_dtypes

    f = np.float32
    bf = ml_dtypes.bfloat16
    f8 = ml_dtypes.float8_e4m3
    q = np.asarray(inputs["q"], f)
    k = np.asarray(inputs["k"], f)
    v = np.asarray(inputs["v"], f)
    qh = np.ascontiguousarray(
        q[:, 0].transpose(1, 0, 2).reshape(FN, BS * SL).astype(bf))
    kh = np.ascontiguousarray(
        k[:, 0].transpose(1, 0, 2).reshape(FN, BS * SL).astype(bf))
    vh = np.ascontiguousarray(
        v[:, 0].transpose(1, 0, 2).reshape(FN, BS * SL).astype(bf))
    W1 = np.asarray(inputs["W1"], f)
    W1p = np.zeros((HIDP, SL * KN), f)
    W1p[:HID] = W1
    # device contraction row ((ic*64+kk)*128+p) = orig col ((ic*128+p)*64+kk)
    W1r = W1p.reshape(HIDP, IC, 128, KN).transpose(1, 3, 2, 0).reshape(
        SL * KN, HIDP)
    # DoubleRow pairs: pair axis = ic -> [m=kk, 128 p, 2 (ic), HIDP]
    W1m = W1r.reshape(IC, KN, 128, HIDP).transpose(1, 2, 0, 3)
    w1sum = np.zeros((HIDP,), f)
    w1sum[:HID] = W1.sum(axis=1)
    b1p = np.zeros((HIDP,), f)
    b1p[:HID] = np.asarray(inputs["b1"], f)
    W2T = np.zeros((HIDP, KN), f)
    W2T[:HID] = np.asarray(inputs["W2"], f).T
    b2 = np.asarray(inputs["b2"], f)
    # leaky-relu token-head indicators: t = c*64 + hl*32 + q*8 + b8
    thead = (np.arange(T) // 32)         # c*2 + hl
    in_maps = []
    for c in range(N_CORES):
        h0 = HL * c
        packf = np.zeros((128, PCW), f)
        packf[:, PC_BQ] = _dup_b(inputs["bq"], c)
        packf[:, PC_BK] = _dup_b(inputs["bk"], c)
        packf[:, PC_BV] = _dup_b(inputs["bv"], c)
        for hl in range(HL):
            packf[KN * hl:KN * (hl + 1), PC_MASK + hl] = 1.0 / KN
            packf[hl, PC_SEL + hl * KN:PC_SEL + (hl + 1) * KN] = 1.0
        packf[0:KN, PC_B2] = b2
        packf[:, PC_ONES] = 1.0 / 128.0
        packf[0, PC_BC1:PC_BC1 + 128] = 1.0
        for hl in range(HL):
            packf[hl, PC_BNP:PC_BNP + 8] = [
                inputs["gq"][h0 + hl], inputs["beq"][h0 + hl],
                inputs["gk"][h0 + hl], inputs["bek"][h0 + hl],
                inputs["gv"][h0 + hl], inputs["bev"][h0 + hl],
                inputs["g1"][h0 + hl], inputs["be1"][h0 + hl],
            ]
        packf[0, PC_BNP1:PC_BNP1 + 4] = [
            inputs["g1"][h0], inputs["be1"][h0],
            inputs["g1"][h0 + 1], inputs["be1"][h0 + 1],
        ]
        packf[0:8, PC_EYE8:PC_EYE8 + 8] = np.eye(8, dtype=f)
        packf[0, PC_WB:PC_WB + 5 * 128] = w1sum[c * HSH:(c + 1) * HSH]
        packf[1, PC_WB:PC_WB + 5 * 128] = b1p[c * HSH:(c + 1) * HSH]
        packb = np.zeros((128, PBW), f)
        packb[:, 0:128] = np.eye(128, dtype=f)
        W2c = W2T[c * HSH:(c + 1) * HSH, :]
        for j in range(5):
            packb[:, PB_W2 + j * KN:PB_W2 + (j + 1) * KN] = \
                W2c[j * 128:(j + 1) * 128, :]
        for hl in range(HL):
            for cc in range(N_CORES):
                head = cc * 2 + hl
                packb[cc, PB_IND + hl * T:PB_IND + (hl + 1) * T] = \
                    (thead == head).astype(f)
        w1c = np.ascontiguousarray(
            (W1m[:, :, :, c * HSH:(c + 1) * HSH] * SCALE_W)
            .transpose(0, 1, 2, 3)
            .reshape(NM, 128, 2 * HSH).astype(f8))
        m = {
            "qh": qh, "kh": kh, "vh": vh,
            "wqT": np.ascontiguousarray(_dup_wT(inputs["Wq"], c).astype(bf)),
            "wkT": np.ascontiguousarray(_dup_wT(inputs["Wk"], c).astype(bf)),
            "wvT": np.ascontiguousarray(_dup_wT(inputs["Wv"], c).astype(bf)),
            "packf": packf,
            "packb": np.ascontiguousarray(packb.astype(bf)),
            "w1a": np.ascontiguousarray(w1c[:NMA]),
            "w1b": np.ascontiguousarray(w1c[NMA:]),
        }
        in_maps.append(m)
    return in_maps


def _unshard(o):
    # out cols: t = c*64 + hl*32 + q*8 + b8;  head = c*2 + hl,  b = q*8+b8
    out = (
        np.asarray(o, np.float32)
        .reshape(KN, N_CORES, HL, BS)
        .transpose(3, 1, 2, 0)
        .reshape(BS, HEADS, KN)[:, None]
    )
    return np.ascontiguousarray(out.astype(np.float32))


def kernel(**inputs):
    global _prog
    if _prog is None:
        _prog = _build()
    from concourse.bass_utils import run_bass_kernel_spmd

    in_maps = _prep_in_maps(inputs)
    res = run_bass_kernel_spmd(_prog, in_maps, list(range(N_CORES)))
    return _unshard(res.results[0]["out"])


# revision 8
# speedup vs baseline: 3.1812x; 1.4336x over previous
"""Trainium2 Bass kernel for nn_MultiHeadAttention_34144990003301 (v5).

Head-parallel attention (2 heads/core), bf16 q/k/v datapath.
BatchNorm1 is POSTPONED past para_linear1: the affine commutes through
the linear layer (h1 = a1*(W1@O) + c1*rowsum(W1) + b1), so raw attention
output O is quantized to fp8e4 (scaled x32), AllGathered in quarters of
the batch DURING attention, and para_linear1 runs as fp8 DoubleRow
matmuls (2 contraction tiles per MM, W1 scaled x2^14). The BN1 bias term
enters as a tiny rank-2 matmul (D) appended to the accumulation; the
per-head scale a1 is applied after W2 (leaky(a*x) = a*leaky(x), a>0).
BN1 stats are AllGathered as 6 floats/core. One AllReduce of the W2
partials, sigmoid on device.

kernel(**inputs) takes the full unsharded inputs, returns [32,1,16,64] f32.
"""

import numpy as np

BS, HEADS, FN, SL, KN, ST = 32, 16, 124, 256, 64, 4
HID = 5000
HIDP = 5120                    # zero-padded hid
EPS = 1e-5
SLOPE = 0.01
N_CORES = 8
HL = HEADS // N_CORES          # 2 local heads per core
ROWS = HL * KN                 # 128 projected rows (per-head 64, duplicated)
T = BS * HEADS                 # 512 global tokens
HSH = HIDP // N_CORES          # 640 hid cols per core (5 blocks of 128)
IC = SL // 128                 # 2 i-chunks
NM = 64                        # DoubleRow kt-pairs (128 kt tiles / 2)
NMA = 48                       # pairs resident early (w1a)
NMB = NM - NMA                 # pairs streamed late (w1b)
NQ = 2                         # AllGather half-chunks
QB = BS // NQ                  # 16 batches per chunk
SCALE_W = 2.0 ** 14            # W1 fp8 scale
SCALE_O = 32.0                 # attention-output fp8 scale
INV_SCALES = 1.0 / (SCALE_W * SCALE_O)
# packed f32 const columns
PC_BQ, PC_BK, PC_BV = 0, 1, 2
PC_MASK = 3                    # 2 cols
PC_B2 = 5
PC_ONES = 6                    # value 1/128
PC_SEL = 7                     # 128 cols (rows 0:2)
PC_BC1 = 135                   # 128 cols (row 0) value 1.0
PC_BNP = 263                   # 8 cols (rows 0:2)
PC_BNP1 = 271                  # 4 cols (row 0)
PC_EYE8 = 275                  # 8 cols (rows 0:8)
PC_WB = 283                    # 640 cols (rows 0:2): w1s | b1 shard
PCW = 923
# packed bf16 cols: eye128 | w2 (5*KN) | ind0 (512) | ind1 (512)
PB_W2 = 128
PB_IND = 128 + 5 * KN
PBW = PB_IND + 2 * T

_prog = None


def _build():
    import concourse.bacc as bacc
    import concourse.tile as tile
    import concourse.mybir as mybir

    f32 = mybir.dt.float32
    bf16 = mybir.dt.bfloat16
    f8 = mybir.dt.float8e4
    AF = mybir.ActivationFunctionType
    OP = mybir.AluOpType
    DR = mybir.MatmulPerfMode.DoubleRow
    RG = [list(range(N_CORES))]

    nc = bacc.Bacc("TRN2", target_bir_lowering=False, debug=False,
                   num_devices=N_CORES)

    def din(name, shape, dt=f32):
        return nc.dram_tensor(
            name, list(shape), dt, kind="ExternalInput"
        ).ap()

    q_d = din("qh", (FN, BS * SL), bf16)
    k_d = din("kh", (FN, BS * SL), bf16)
    v_d = din("vh", (FN, BS * SL), bf16)
    wq_d = din("wqT", (FN, ROWS), bf16)
    wk_d = din("wkT", (FN, ROWS), bf16)
    wv_d = din("wvT", (FN, ROWS), bf16)
    pk_d = din("packf", (128, PCW))
    pb_d = din("packb", (128, PBW), bf16)
    w1a_d = din("w1a", (NMA, 128, 2 * HSH), f8)
    w1b_d = din("w1b", (NMB, 128, 2 * HSH), f8)
    out_d = nc.dram_tensor("out", [KN, T], f32, kind="ExternalOutput").ap()

    with tile.TileContext(nc) as tc:
        with (
            tc.tile_pool(name="persist", bufs=1) as pp,
            tc.tile_pool(name="dram", bufs=1, space="DRAM") as dp,
        ):
            pk_sb = pp.tile([128, PCW], f32, tag="packf")
            pb_sb = pp.tile([128, PBW], bf16, tag="packb")
            w1a = pp.tile([128, NMA * 2 * HSH], f8, tag="w1a")
            v1 = pp.tile([128, IC * KN * T], f8, tag="v1")
            O_all = pp.tile([128, NQ * 4096], f8, tag="oall")
            ab_sb = pp.tile([128, 6], f32, tag="absb")

            bq_sb = pk_sb[:, PC_BQ:PC_BQ + 1]
            bk_sb = pk_sb[:, PC_BK:PC_BK + 1]
            bv_sb = pk_sb[:, PC_BV:PC_BV + 1]
            mask_sb = pk_sb[:, PC_MASK:PC_MASK + 2]
            b2_sb = pk_sb[0:KN, PC_B2:PC_B2 + 1]
            ones128 = pk_sb[:, PC_ONES:PC_ONES + 1]      # value 1/128
            sel_sb = pk_sb[0:HL, PC_SEL:PC_SEL + 128]
            bc1_sb = pk_sb[0:1, PC_BC1:PC_BC1 + 128]
            bnp_sb = pk_sb[0:HL, PC_BNP:PC_BNP + 8]
            bnp1_sb = pk_sb[0:1, PC_BNP1:PC_BNP1 + 4]
            eye8_sb = pk_sb[0:8, PC_EYE8:PC_EYE8 + 8]
            wb_sb = pk_sb[0:2, PC_WB:PC_WB + 5 * 128]
            eye_sb = pb_sb[:, 0:128]
            w2_sb = [pb_sb[:, PB_W2 + j * KN:PB_W2 + (j + 1) * KN]
                     for j in range(5)]
            ind_sb = [pb_sb[0:8, PB_IND + h * T:PB_IND + (h + 1) * T]
                      for h in range(HL)]

            nc.sync.dma_start(pk_sb[:], pk_d)
            nc.scalar.dma_start(pb_sb[:], pb_d)
            w1av = w1a[:].rearrange("p (m x) -> p m x", m=NMA)
            for ch in range(3):
                nc.scalar.dma_start(
                    w1av[:, 16 * ch:16 * (ch + 1), :],
                    w1a_d[16 * ch:16 * (ch + 1)].transpose([1, 0, 2]),
                )
            # prefetch natural_log_exp act table
            dummy = pp.tile([1, 1], f32, tag="dummy")
            nc.scalar.activation(dummy[:], ones128[0:1, :], AF.Ln,
                                 bias=0.0, scale=1.0)

            # attention-output layout (blocks of (ic,k,hl,b16) per half):
            # col = h*4096 + ic*2048 + k*32 + hl*16 + b16
            Ov = O_all[:].rearrange("p (q a k l b) -> p q a k l b",
                                    q=NQ, a=IC, k=KN, l=HL)
            # gathered layout: col = (h*8+c)*4096 + ic*2048 + k*32
            #                        + hl*16 + b16
            v1r = v1[:].rearrange("p (b a k r) -> p b a k r",
                                  b=16, a=IC, k=KN)

            fgls = []
            with tc.tile_pool(name="proj", bufs=1) as prp:
                qp = prp.tile([ROWS, BS * SL], bf16, tag="qp")
                kp = prp.tile([ROWS, BS * SL], bf16, tag="kp")
                vp = prp.tile([ROWS, BS * SL], bf16, tag="vp")

                # ---------- Phase A: qp/kp/vp = W[R,:] @ x + b ----------
                with (
                    tc.tile_pool(name="xin", bufs=3) as xp,
                    tc.tile_pool(name="wts", bufs=1) as wp,
                    tc.tile_pool(name="psA", bufs=3, space="PSUM") as psA,
                    tc.tile_pool(name="stat", bufs=1) as st,
                ):
                    ws = []
                    for ti, w_d in enumerate((wq_d, wk_d, wv_d)):
                        w_sb = wp.tile([FN, ROWS], bf16, tag=f"w{ti}",
                                       name=f"w{ti}")
                        nc.sync.dma_start(w_sb[:], w_d)
                        ws.append(w_sb)
                    bnsts = [
                        st.tile([ROWS, 16 * 6], f32, tag=f"bnst{ti}",
                                name=f"bnst{ti}")
                        for ti in range(3)
                    ]
                    for ti, (x_d, b_sb, dst) in enumerate(
                        ((q_d, bq_sb, qp), (k_d, bk_sb, kp),
                         (v_d, bv_sb, vp))
                    ):
                        for xc in range(4):
                            xcs = slice(xc * 2048, (xc + 1) * 2048)
                            x_sb = xp.tile([FN, 2048], bf16, tag="xch",
                                           name=f"x{ti}_{xc}")
                            nc.sync.dma_start(x_sb[:], x_d[:, xcs])
                            for n in range(4):
                                cs = slice(xc * 2048 + n * 512,
                                           xc * 2048 + (n + 1) * 512)
                                ncs = 4 * xc + n
                                ps = psA.tile([ROWS, 512], f32, tag="proj",
                                              name=f"proj{ti}_{ncs}")
                                nc.tensor.matmul(
                                    ps[:], ws[ti][:],
                                    x_sb[:, n * 512:(n + 1) * 512])
                                nc.scalar.activation(
                                    dst[:, cs], ps[:], AF.Identity,
                                    bias=b_sb, scale=1.0,
                                )
                                nc.vector.bn_stats(
                                    bnsts[ti][:, 6 * ncs:6 * (ncs + 1)],
                                    dst[:, cs],
                                )

                    # ---------- Phase B: per-head BN affine for q/k/v ----
                    with tc.tile_pool(name="psB", bufs=1,
                                      space="PSUM") as psB:
                        AB = st.tile([HL, 6], f32, tag="AB")
                        for ti in range(3):
                            gc, bc_ = 2 * ti, 2 * ti + 1
                            mv = st.tile([ROWS, 2], f32, tag=f"mv{ti}",
                                         name=f"mv{ti}")
                            nc.vector.bn_aggr(
                                mv[:],
                                bnsts[ti][:].rearrange(
                                    "p (c s) -> p c s", s=6
                                ),
                            )
                            stat2 = st.tile([ROWS, 2], f32, tag=f"s2{ti}",
                                            name=f"s2{ti}")
                            nc.vector.tensor_copy(stat2[:, 0:1], mv[:, 0:1])
                            nc.vector.scalar_tensor_tensor(
                                stat2[:, 1:2], mv[:, 0:1], mv[:, 0:1],
                                mv[:, 1:2], op0=OP.mult, op1=OP.add,
                            )
                            hs = psB.tile([HL, 2], f32, tag=f"hs{ti}",
                                          name=f"hs{ti}")
                            nc.tensor.matmul(hs[:], mask_sb, stat2[:])
                            mean_h = st.tile([HL, 1], f32, tag=f"mh{ti}",
                                             name=f"mh{ti}")
                            nc.vector.tensor_copy(mean_h[:], hs[:, 0:1])
                            tmp = st.tile([HL, 1], f32, tag=f"tp{ti}",
                                          name=f"tp{ti}")
                            nc.vector.tensor_tensor(
                                tmp[:], mean_h[:], mean_h[:], op=OP.mult
                            )
                            var_h = st.tile([HL, 1], f32, tag=f"vh{ti}",
                                            name=f"vh{ti}")
                            nc.vector.tensor_tensor(
                                var_h[:], hs[:, 1:2], tmp[:],
                                op=OP.subtract,
                            )
                            nc.vector.tensor_scalar_add(
                                var_h[:], var_h[:], EPS
                            )
                            lnv = st.tile([HL, 1], f32, tag=f"ln{ti}",
                                          name=f"ln{ti}")
                            nc.scalar.activation(lnv[:], var_h[:], AF.Ln,
                                                 bias=0.0, scale=1.0)
                            rsq = st.tile([HL, 1], f32, tag=f"rq{ti}",
                                          name=f"rq{ti}")
                            nc.scalar.activation(rsq[:], lnv[:], AF.Exp,
                                                 bias=0.0, scale=-0.5)
                            a_h = st.tile([HL, 1], f32, tag=f"ah{ti}",
                                          name=f"ah{ti}")
                            nc.vector.tensor_tensor(
                                a_h[:], bnp_sb[:, gc:gc + 1], rsq[:],
                                op=OP.mult,
                            )
                            tmp2 = st.tile([HL, 1], f32, tag=f"t2{ti}",
                                           name=f"t2{ti}")
                            nc.vector.tensor_tensor(
                                tmp2[:], mean_h[:], a_h[:], op=OP.mult
                            )
                            nc.vector.tensor_tensor(
                                AB[:, bc_:bc_ + 1], bnp_sb[:, bc_:bc_ + 1],
                                tmp2[:], op=OP.subtract,
                            )
                            nc.vector.tensor_copy(AB[:, gc:gc + 1], a_h[:])
                        bc_ps = psB.tile([128, 6], f32, tag="bcps")
                        nc.tensor.matmul(bc_ps[:], sel_sb, AB[:])
                        nc.vector.tensor_copy(ab_sb[:], bc_ps[:])

                # ---------- Phase C: attention, 2 heads, AG quarters ----
                with (
                    tc.tile_pool(name="stage", bufs=3) as sg,
                    tc.tile_pool(name="expp", bufs=2) as epool,
                    tc.tile_pool(name="vwp", bufs=3) as vwp,
                    tc.tile_pool(name="small", bufs=4) as smp,
                    tc.tile_pool(name="ps_sc", bufs=2, space="PSUM") as pssc,
                    tc.tile_pool(name="ps_vt", bufs=2, space="PSUM") as psvt,
                    tc.tile_pool(name="ps_uo", bufs=2, space="PSUM") as psuo,
                ):
                    for b in range(BS):
                        qq, b8 = divmod(b, QB)
                        bsl = slice(b * SL, (b + 1) * SL)
                        qw2 = sg.tile([128, SL], bf16, tag="qw")
                        nc.gpsimd.tensor_scalar(
                            qw2[:], qp[:, bsl], ab_sb[:, 0:1], ab_sb[:, 1:2],
                            op0=OP.mult, op1=OP.add,
                        )
                        kw2 = sg.tile([128, SL], bf16, tag="kw")
                        nc.gpsimd.tensor_scalar(
                            kw2[:], kp[:, bsl], ab_sb[:, 2:3], ab_sb[:, 3:4],
                            op0=OP.mult, op1=OP.add,
                        )
                        vw2 = sg.tile([128, SL], bf16, tag="vw")
                        nc.vector.tensor_scalar(
                            vw2[:], vp[:, bsl], ab_sb[:, 4:5], ab_sb[:, 5:6],
                            op0=OP.mult, op1=OP.add,
                        )
                        # scores both heads: [128(j in jc), hl*512 + i]
                        sc_ps = pssc.tile([128, 1024], f32, tag="scps")
                        for hl in range(HL):
                            r = slice(KN * hl, KN * (hl + 1))
                            for jc in range(2):
                                nc.tensor.matmul(
                                    sc_ps[:, hl * 512 + jc * 256:
                                          hl * 512 + (jc + 1) * 256],
                                    kw2[r, jc * 128:(jc + 1) * 128],
                                    qw2[r, :],
                                )
                        eT = epool.tile([128, 1024], bf16, tag="expT")
                        nc.scalar.activation(
                            eT[:], sc_ps[:], AF.Exp, bias=0.0, scale=0.125,
                        )
                        # vw transposed: [128(s in jc), k both heads]
                        vt_ps = psvt.tile([128, 256], bf16, tag="vtps")
                        for jc in range(2):
                            nc.tensor.transpose(
                                vt_ps[:, jc * 128:(jc + 1) * 128],
                                vw2[:, jc * 128:(jc + 1) * 128],
                                eye_sb,
                            )
                        vws2 = vwp.tile([128, 2 * 2 * (KN + 1)], bf16,
                                        tag="vws")
                        vws2v = vws2[:].rearrange(
                            "p (a h e) -> p a h e", a=2, h=2
                        )
                        for jc in range(2):
                            nc.vector.tensor_copy(
                                vws2v[:, jc, :, 0:KN],
                                vt_ps[:, jc * 128:(jc + 1) * 128].rearrange(
                                    "p (h e) -> p h e", h=2
                                ),
                            )
                        nc.vector.memset(vws2v[:, :, :, KN:KN + 1],
                                         1.0 / SCALE_O)
                        # unnormalized o + scaled exp row sums (col KN)
                        uo = psuo.tile([128, 2 * 2 * (KN + 1)], f32,
                                       tag="uo")
                        for hl in range(HL):
                            for ic in range(IC):
                                c0 = hl * 130 + ic * 65
                                for jc in range(2):
                                    nc.tensor.matmul(
                                        uo[:, c0:c0 + KN + 1],
                                        eT[:, hl * 512 + jc * 256 + ic * 128:
                                           hl * 512 + jc * 256 +
                                           (ic + 1) * 128],
                                        vws2v[:, jc, hl, :],
                                        start=(jc == 0), stop=(jc == 1),
                                    )
                        rec = smp.tile([128, 4], f32, tag="rec")
                        nc.vector.reciprocal(
                            rec[:].rearrange("p (h i e) -> p h i e",
                                             h=2, i=2),
                            uo[:].rearrange("p (h i e) -> p h i e",
                                            h=2, i=2)[:, :, :, KN:KN + 1],
                        )
                        for hl in range(HL):
                            for ic in range(IC):
                                c0 = hl * 130 + ic * 65
                                dst = Ov[:, qq, ic, :, hl, b8]
                                rc = rec[:, 2 * hl + ic:2 * hl + ic + 1]
                                if ic == 0 and hl == 0:
                                    nc.scalar.activation(
                                        dst, uo[:, c0:c0 + KN], AF.Identity,
                                        bias=0.0, scale=rc,
                                    )
                                else:
                                    nc.vector.tensor_scalar(
                                        dst, uo[:, c0:c0 + KN], rc, None,
                                        op0=OP.mult,
                                    )
                        if b8 == QB - 1:
                            floc = dp.tile([128, 4096], f8,
                                           tag=f"floc{qq}",
                                           name=f"floc{qq}")
                            nc.sync.dma_start(
                                floc[:],
                                O_all[:, qq * 4096:(qq + 1) * 4096],
                            )
                            fgl = dp.tile([N_CORES, 128, 4096], f8,
                                          tag=f"fgl{qq}", name=f"fgl{qq}",
                                          addr_space="Shared")
                            nc.gpsimd.collective_compute(
                                "AllGather", OP.bypass, replica_groups=RG,
                                ins=[floc[:].opt()], outs=[fgl[:].opt()],
                            )
                            fgls.append(fgl)
                            for c in range(N_CORES):
                                blk = (qq * 8 + c) * 4096
                                eng = nc.sync if c % 2 else nc.scalar
                                eng.dma_start(
                                    v1[:, blk:blk + 4096], fgl[c][:, :],
                                )

            # ---------- Phase F: fp8 DoubleRow para_linear1 ----------
            with (
                tc.tile_pool(name="pf", bufs=1) as fp,
                tc.tile_pool(name="scrp", bufs=2) as scp,
                tc.tile_pool(name="st1", bufs=1) as st1,
                tc.tile_pool(name="psH", bufs=1, space="PSUM") as psH,
                tc.tile_pool(name="psD", bufs=2, space="PSUM") as psD,
            ):
                w1b = fp.tile([128, NMB * 2 * HSH], f8, tag="w1b")
                w1bv = w1b[:].rearrange("p (m x) -> p m x", m=NMB)
                nc.scalar.dma_start(w1bv[:], w1b_d.transpose([1, 0, 2]))
                w1am = w1a[:].rearrange("p (m t j) -> p m t j",
                                        m=NMA, t=2)
                w1bm = w1b[:].rearrange("p (m t j) -> p m t j",
                                        m=NMB, t=2)
                h1ps = [
                    psH.tile([128, T], f32, tag=f"h1_{j}", name=f"h1ps{j}")
                    for j in range(5)
                ]
                for m in range(NM):
                    lt = w1am[:, m] if m < NMA else w1bm[:, m - NMA]
                    rhs = v1r[:, :, :, m, :].transpose([0, 2, 1, 3])
                    for j in range(5):
                        nc.tensor.matmul(
                            h1ps[j][:], lt[:, :, j * 128:(j + 1) * 128],
                            rhs, start=(m == 0), stop=False,
                            perf_mode=DR, skip_group_check=True,
                        )

                # ---------- BN1 stats (local heads) ----------
                Os = O_all[:].rearrange("p (x l b) -> p x l b",
                                        l=HL, b=16)
                st2 = st1.tile([128, 4], f32, tag="st2")
                for hl in range(HL):
                    npe = NQ * IC * KN * QB      # 4096 elems/partition
                    scrap = scp.tile([128, npe], bf16, tag="scrap",
                                     name=f"scrap{hl}")
                    sum1 = st1.tile([128, 1], f32, tag=f"sum{hl}")
                    nc.vector.tensor_scalar(
                        scrap[:].rearrange("p (x b) -> p x b", b=16),
                        Os[:, :, hl, :], 1.0, None, op0=OP.mult,
                        op1=OP.add, accum_out=sum1[:],
                    )
                    scrap2 = scp.tile([128, npe], bf16, tag="scrap",
                                      name=f"scrap2{hl}")
                    sq1 = st1.tile([128, 1], f32, tag=f"sq{hl}")
                    nc.scalar.activation(
                        scrap2[:].rearrange("p (x b) -> p x b", b=16),
                        Os[:, :, hl, :], AF.Square, accum_out=sq1[:],
                    )
                    nc.vector.tensor_scalar_mul(
                        st2[:, 2 * hl:2 * hl + 1], sum1[:], 1.0 / npe
                    )
                    nc.vector.tensor_scalar_mul(
                        st2[:, 2 * hl + 1:2 * hl + 2], sq1[:], 1.0 / npe
                    )
                hs1 = psD.tile([1, 4], f32, tag="psd", name="hs1")
                nc.tensor.matmul(hs1[:], ones128, st2[:])
                hsb = st1.tile([1, 4], f32, tag="hsb")
                nc.vector.tensor_copy(hsb[:], hs1[:])
                # local affine params -> [A_hl0, B_hl0, A_hl1, B_hl1,
                #                         a1'_hl0, a1'_hl1]
                arst = st1.tile([1, 6], f32, tag="arst")
                for hl in range(HL):
                    ms = hsb[:, 2 * hl:2 * hl + 1]       # 32*mean
                    qs = hsb[:, 2 * hl + 1:2 * hl + 2]   # 1024*E[x^2]
                    m2 = st1.tile([1, 1], f32, tag=f"m2_{hl}")
                    nc.vector.tensor_tensor(m2[:], ms, ms, op=OP.mult)
                    v32 = st1.tile([1, 1], f32, tag=f"v32_{hl}")
                    nc.vector.tensor_tensor(v32[:], qs, m2[:],
                                            op=OP.subtract)
                    varp = st1.tile([1, 1], f32, tag=f"vp_{hl}")
                    nc.vector.tensor_scalar(
                        varp[:], v32[:], 1.0 / (SCALE_O * SCALE_O), EPS,
                        op0=OP.mult, op1=OP.add,
                    )
                    lnv = st1.tile([1, 1], f32, tag=f"lnv_{hl}")
                    nc.scalar.activation(lnv[:], varp[:], AF.Ln,
                                         bias=0.0, scale=1.0)
                    rs = st1.tile([1, 1], f32, tag=f"rs_{hl}")
                    nc.scalar.activation(rs[:], lnv[:], AF.Exp,
                                         bias=0.0, scale=-0.5)
                    a1 = st1.tile([1, 1], f32, tag=f"a1_{hl}")
                    nc.vector.tensor_tensor(
                        a1[:], bnp1_sb[:, 2 * hl:2 * hl + 1], rs[:],
                        op=OP.mult,
                    )
                    inva = st1.tile([1, 1], f32, tag=f"ia_{hl}")
                    nc.vector.reciprocal(inva[:], a1[:])
                    mm = st1.tile([1, 1], f32, tag=f"mm_{hl}")
                    nc.vector.tensor_scalar_mul(mm[:], ms, 1.0 / SCALE_O)
                    am = st1.tile([1, 1], f32, tag=f"am_{hl}")
                    nc.vector.tensor_tensor(am[:], a1[:], mm[:],
                                            op=OP.mult)
                    c1 = st1.tile([1, 1], f32, tag=f"c1_{hl}")
                    nc.vector.tensor_tensor(
                        c1[:], bnp1_sb[:, 2 * hl + 1:2 * hl + 2], am[:],
                        op=OP.subtract,
                    )
                    ci = st1.tile([1, 1], f32, tag=f"ci_{hl}")
                    nc.vector.tensor_tensor(ci[:], c1[:], inva[:],
                                            op=OP.mult)
                    nc.vector.tensor_scalar_mul(
                        arst[:, 2 * hl:2 * hl + 1], ci[:],
                        SCALE_W * SCALE_O,
                    )
                    nc.vector.tensor_scalar_mul(
                        arst[:, 2 * hl + 1:2 * hl + 2], inva[:],
                        SCALE_W * SCALE_O,
                    )
                    nc.vector.tensor_scalar_mul(
                        arst[:, 4 + hl:5 + hl], a1[:], INV_SCALES,
                    )
                arst_d = dp.tile([1, 6], f32, tag="arstd")
                nc.sync.dma_start(arst_d[:], arst[:])
                absh = dp.tile([N_CORES, 1, 6], f32, tag="absh",
                               addr_space="Shared")
                nc.gpsimd.collective_compute(
                    "AllGather", OP.bypass, replica_groups=RG,
                    ins=[arst_d[:].opt()], outs=[absh[:].opt()],
                )
                ab8 = st1.tile([N_CORES, 6], f32, tag="ab8")
                nc.sync.dma_start(ab8[:], absh[:, 0, :])
                # transpose per-head params to rows
                abT = []
                for s in range(2):
                    tp = psD.tile([2, N_CORES], f32, tag="psd",
                                  name=f"abTp{s}")
                    nc.tensor.transpose(tp[:], ab8[:, 2 * s:2 * s + 2],
                                        eye8_sb)
                    sb = st1.tile([2, N_CORES], f32, tag=f"abT{s}")
                    nc.vector.tensor_copy(sb[:], tp[:])
                    abT.append(sb)
                a1s = []
                for hl in range(HL):
                    tp = psD.tile([1, N_CORES], f32, tag="psd",
                                  name=f"a1Tp{hl}")
                    nc.tensor.transpose(tp[:], ab8[:, 4 + hl:5 + hl],
                                        eye8_sb)
                    sb = st1.tile([1, N_CORES], f32, tag=f"a1s{hl}")
                    nc.vector.tensor_copy(sb[:], tp[:])
                    a1s.append(sb)
                # D rows: dts[(j,hl)] = [8c, 128p] = A_hl[c]*w1s + B_hl[c]*b1
                dts = st1.tile([8, 10 * 128], bf16, tag="dts")
                for j in range(5):
                    for hl in range(HL):
                        dtp = psD.tile([8, 128], f32, tag="psd",
                                       name=f"dtp{j}_{hl}")
                        nc.tensor.matmul(
                            dtp[:], abT[hl][:],
                            wb_sb[:, j * 128:(j + 1) * 128],
                        )
                        nc.vector.tensor_copy(
                            dts[:, (j * 2 + hl) * 128:
                                (j * 2 + hl + 1) * 128],
                            dtp[:],
                        )
                # a1 broadcast [64, 16] (col = hl*8 + c)
                a1bc = st1.tile([KN, 16], f32, tag="a1bc")
                for hl in range(HL):
                    a1p = psD.tile([KN, N_CORES], f32, tag="psd",
                                   name=f"a1p{hl}")
                    nc.tensor.matmul(a1p[:], bc1_sb[:, 0:KN], a1s[hl][:])
                    nc.vector.tensor_copy(
                        a1bc[:, hl * 8:(hl + 1) * 8], a1p[:]
                    )
                # D-add into h1 psums
                for j in range(5):
                    for hl in range(HL):
                        nc.tensor.matmul(
                            h1ps[j][:],
                            dts[:, (j * 2 + hl) * 128:
                                (j * 2 + hl + 1) * 128],
                            ind_sb[hl],
                            start=False, stop=(hl == HL - 1),
                            skip_group_check=True,
                        )

                # ---------- leaky + W2 + scale + AllReduce + sigmoid ----
                h1sb = [
                    fp.tile([128, T], bf16, tag=f"h1s_{j}", name=f"h1s{j}")
                    for j in range(5)
                ]
                h1af = [
                    fp.tile([128, T], bf16, tag=f"h1a_{j}", name=f"h1a{j}")
                    for j in range(5)
                ]
                for j in range(5):
                    nc.scalar.activation(
                        h1af[j][:], h1ps[j][:], AF.Identity,
                        bias=0.0, scale=1.0,
                    )
                    nc.vector.scalar_tensor_tensor(
                        h1sb[j][:], h1af[j][:], SLOPE, h1af[j][:],
                        op0=OP.mult, op1=OP.max,
                    )
                ps2 = psH.tile([KN, T], f32, tag="out2")
                for j in range(5):
                    nc.tensor.matmul(
                        ps2[:], w2_sb[j], h1sb[j][:],
                        start=(j == 0), stop=(j == 4),
                    )
                o2f = fp.tile([KN, T], f32, tag="o2f")
                nc.vector.tensor_copy(o2f[:], ps2[:])
                o2s = fp.tile([KN, T], f32, tag="o2s")
                for h in range(NQ):
                    for c in range(N_CORES):
                        for hl in range(HL):
                            g = h * 256 + c * 32 + hl * 16
                            nc.vector.tensor_scalar(
                                o2s[:, g:g + 16], o2f[:, g:g + 16],
                                a1bc[:, hl * 8 + c:hl * 8 + c + 1], None,
                                op0=OP.mult,
                            )
                arin = dp.tile([KN, T], f32, tag="arin")
                nc.sync.dma_start(arin[:], o2s[:])
                arout = dp.tile([KN, T], f32, tag="arout",
                                addr_space="Shared")
                nc.gpsimd.collective_compute(
                    "AllReduce", OP.add, replica_groups=RG,
                    ins=[arin[:].opt()], outs=[arout[:].opt()],
                )
                arsb = fp.tile([KN, T], f32, tag="arsb")
                nc.sync.dma_start(arsb[:], arout[:])
                fin = fp.tile([KN, T], f32, tag="fin")
                nc.scalar.activation(
                    fin[:], arsb[:], AF.Sigmoid, bias=b2_sb, scale=1.0
                )
                nc.sync.dma_start(out_d, fin[:])

    nc.compile()
    return nc


def _dup_wT(W, c):
    W = np.asarray(W, np.float32)
    cols = [W[8 * c + ST * hl: 8 * c + ST * hl + KN, :].T for hl in range(HL)]
    return np.concatenate(cols, axis=1)


def _dup_b(b, c):
    b = np.asarray(b, np.float32)
    rows = [b[8 * c + ST * hl: 8 * c + ST * hl + KN] for hl in range(HL)]
    return np.ascontiguousarray(np.concatenate(rows))


def _prep_in_maps(inputs):
    import ml_dtypes

    f = np.float32
    bf = ml_dtypes.bfloat16
    f8 = ml_dtypes.float8_e4m3
    q = np.asarray(inputs["q"], f)
    k = np.asarray(inputs["k"], f)
    v = np.asarray(inputs["v"], f)
    qh = np.ascontiguousarray(
        q[:, 0].transpose(1, 0, 2).reshape(FN, BS * SL).astype(bf))
    kh = np.ascontiguousarray(
        k[:, 0].transpose(1, 0, 2).reshape(FN, BS * SL).astype(bf))
    vh = np.ascontiguousarray(
        v[:, 0].transpose(1, 0, 2).reshape(FN, BS * SL).astype(bf))
    W1 = np.asarray(inputs["W1"], f)
    W1p = np.zeros((HIDP, SL * KN), f)
    W1p[:HID] = W1
    # device contraction row ((ic*64+kk)*128+p) = orig col ((ic*128+p)*64+kk)
    W1r = W1p.reshape(HIDP, IC, 128, KN).transpose(1, 3, 2, 0).reshape(
        SL * KN, HIDP)
    # DoubleRow pairs: pair axis = ic -> [m=kk, 128 p, 2 (ic), HIDP]
    W1m = W1r.reshape(IC, KN, 128, HIDP).transpose(1, 2, 0, 3)
    w1sum = np.zeros((HIDP,), f)
    w1sum[:HID] = W1.sum(axis=1)
    b1p = np.zeros((HIDP,), f)
    b1p[:HID] = np.asarray(inputs["b1"], f)
    W2T = np.zeros((HIDP, KN), f)
    W2T[:HID] = np.asarray(inputs["W2"], f).T
    b2 = np.asarray(inputs["b2"], f)
    # token order: t = h*256 + c*32 + hl*16 + b16; head = c*2 + hl
    tt = np.arange(T)
    thead = 2 * ((tt % 256) // 32) + (tt % 32) // 16
    in_maps = []
    for c in range(N_CORES):
        h0 = HL * c
        packf = np.zeros((128, PCW), f)
        packf[:, PC_BQ] = _dup_b(inputs["bq"], c)
        packf[:, PC_BK] = _dup_b(inputs["bk"], c)
        packf[:, PC_BV] = _dup_b(inputs["bv"], c)
        for hl in range(HL):
            packf[KN * hl:KN * (hl + 1), PC_MASK + hl] = 1.0 / KN
            packf[hl, PC_SEL + hl * KN:PC_SEL + (hl + 1) * KN] = 1.0
        packf[0:KN, PC_B2] = b2
        packf[:, PC_ONES] = 1.0 / 128.0
        packf[0, PC_BC1:PC_BC1 + 128] = 1.0
        for hl in range(HL):
            packf[hl, PC_BNP:PC_BNP + 8] = [
                inputs["gq"][h0 + hl], inputs["beq"][h0 + hl],
                inputs["gk"][h0 + hl], inputs["bek"][h0 + hl],
                inputs["gv"][h0 + hl], inputs["bev"][h0 + hl],
                inputs["g1"][h0 + hl], inputs["be1"][h0 + hl],
            ]
        packf[0, PC_BNP1:PC_BNP1 + 4] = [
            inputs["g1"][h0], inputs["be1"][h0],
            inputs["g1"][h0 + 1], inputs["be1"][h0 + 1],
        ]
        packf[0:8, PC_EYE8:PC_EYE8 + 8] = np.eye(8, dtype=f)
        packf[0, PC_WB:PC_WB + 5 * 128] = w1sum[c * HSH:(c + 1) * HSH]
        packf[1, PC_WB:PC_WB + 5 * 128] = b1p[c * HSH:(c + 1) * HSH]
        packb = np.zeros((128, PBW), f)
        packb[:, 0:128] = np.eye(128, dtype=f)
        W2c = W2T[c * HSH:(c + 1) * HSH, :]
        for j in range(5):
            packb[:, PB_W2 + j * KN:PB_W2 + (j + 1) * KN] = \
                W2c[j * 128:(j + 1) * 128, :]
        for hl in range(HL):
            for cc in range(N_CORES):
                head = cc * 2 + hl
                packb[cc, PB_IND + hl * T:PB_IND + (hl + 1) * T] = \
                    (thead == head).astype(f)
        w1c = np.ascontiguousarray(
            (W1m[:, :, :, c * HSH:(c + 1) * HSH] * SCALE_W)
            .transpose(0, 1, 2, 3)
            .reshape(NM, 128, 2 * HSH).astype(f8))
        m = {
            "qh": qh, "kh": kh, "vh": vh,
            "wqT": np.ascontiguousarray(_dup_wT(inputs["Wq"], c).astype(bf)),
            "wkT": np.ascontiguousarray(_dup_wT(inputs["Wk"], c).astype(bf)),
            "wvT": np.ascontiguousarray(_dup_wT(inputs["Wv"], c).astype(bf)),
            "packf": packf,
            "packb": np.ascontiguousarray(packb.astype(bf)),
            "w1a": np.ascontiguousarray(w1c[:NMA]),
            "w1b": np.ascontiguousarray(w1c[NMA:]),
        }
        in_maps.append(m)
    return in_maps


def _unshard(o):
    # out cols: t = h*256 + c*32 + hl*16 + b16; head = c*2+hl, b = h*16+b16
    out = (
        np.asarray(o, np.float32)
        .reshape(KN, NQ, N_CORES, HL, QB)
        .transpose(1, 4, 2, 3, 0)
        .reshape(BS, HEADS, KN)[:, None]
    )
    return np.ascontiguousarray(out.astype(np.float32))


def kernel(**inputs):
    global _prog
    if _prog is None:
        _prog = _build()
    from concourse.bass_utils import run_bass_kernel_spmd

    in_maps = _prep_in_maps(inputs)
    res = run_bass_kernel_spmd(_prog, in_maps, list(range(N_CORES)))
    return _unshard(res.results[0]["out"])


# revision 9
# speedup vs baseline: 3.3772x; 1.0616x over previous
"""Trainium2 Bass kernel for nn_MultiHeadAttention_34144990003301 (v5).

Head-parallel attention (2 heads/core), bf16 q/k/v datapath.
BatchNorm1 is POSTPONED past para_linear1: the affine commutes through
the linear layer (h1 = a1*(W1@O) + c1*rowsum(W1) + b1), so raw attention
output O is quantized to fp8e4 (scaled x32), AllGathered in quarters of
the batch DURING attention, and para_linear1 runs as fp8 DoubleRow
matmuls (2 contraction tiles per MM, W1 scaled x2^14). The BN1 bias term
enters as a tiny rank-2 matmul (D) appended to the accumulation; the
per-head scale a1 is applied after W2 (leaky(a*x) = a*leaky(x), a>0).
BN1 stats are AllGathered as 6 floats/core. One AllReduce of the W2
partials, sigmoid on device.

kernel(**inputs) takes the full unsharded inputs, returns [32,1,16,64] f32.
"""

import numpy as np

BS, HEADS, FN, SL, KN, ST = 32, 16, 124, 256, 64, 4
HID = 5000
HIDP = 5120                    # zero-padded hid
EPS = 1e-5
SLOPE = 0.01
N_CORES = 8
HL = HEADS // N_CORES          # 2 local heads per core
ROWS = HL * KN                 # 128 projected rows (per-head 64, duplicated)
T = BS * HEADS                 # 512 global tokens
HSH = HIDP // N_CORES          # 640 hid cols per core (5 blocks of 128)
IC = SL // 128                 # 2 i-chunks
NM = 64                        # DoubleRow kt-pairs (128 kt tiles / 2)
NMA = 48                       # pairs resident early (w1a)
NMB = NM - NMA                 # pairs streamed late (w1b)
NQ = 2                         # AllGather half-chunks
QB = BS // NQ                  # 16 batches per chunk
SCALE_W = 2.0 ** 14            # W1 fp8 scale
SCALE_O = 32.0                 # attention-output fp8 scale
INV_SCALES = 1.0 / (SCALE_W * SCALE_O)
# packed f32 const columns
PC_BQ, PC_BK, PC_BV = 0, 1, 2
PC_MASK = 3                    # 2 cols
PC_B2 = 5
PC_ONES = 6                    # value 1/128
PC_SEL = 7                     # 128 cols (rows 0:2)
PC_BC1 = 135                   # 128 cols (row 0) value 1.0
PC_BNP = 263                   # 8 cols (rows 0:2)
PC_BNP1 = 271                  # 4 cols (row 0)
PC_EYE8 = 275                  # 8 cols (rows 0:8)
PC_WB = 283                    # 640 cols (rows 0:2): w1s | b1 shard
PC_MAGIC = 923                 # 1 col rows 0:2: quake-rsqrt magic bits
PCW = 924
# packed bf16 cols: eye128 | w2 (5*KN) | ind0 (512) | ind1 (512)
PB_W2 = 128
PB_IND = 128 + 5 * KN
PBW = PB_IND + 2 * T

_prog = None


def _build():
    import concourse.bacc as bacc
    import concourse.tile as tile
    import concourse.mybir as mybir

    f32 = mybir.dt.float32
    bf16 = mybir.dt.bfloat16
    f8 = mybir.dt.float8e4
    AF = mybir.ActivationFunctionType
    OP = mybir.AluOpType
    DR = mybir.MatmulPerfMode.DoubleRow
    RG = [list(range(N_CORES))]

    i32 = mybir.dt.int32

    nc = bacc.Bacc("TRN2", target_bir_lowering=False, debug=False,
                   num_devices=N_CORES)

    def din(name, shape, dt=f32):
        return nc.dram_tensor(
            name, list(shape), dt, kind="ExternalInput"
        ).ap()

    q_d = din("qh", (FN, BS * SL), bf16)
    k_d = din("kh", (FN, BS * SL), bf16)
    v_d = din("vh", (FN, BS * SL), bf16)
    wq_d = din("wqT", (FN, ROWS), bf16)
    wk_d = din("wkT", (FN, ROWS), bf16)
    wv_d = din("wvT", (FN, ROWS), bf16)
    pk_d = din("packf", (128, PCW))
    pb_d = din("packb", (128, PBW), bf16)
    w1a_d = din("w1a", (NMA, 128, 2 * HSH), f8)
    w1b_d = din("w1b", (NMB, 128, 2 * HSH), f8)
    out_d = nc.dram_tensor("out", [KN, T], f32, kind="ExternalOutput").ap()

    with tile.TileContext(nc) as tc:
        with (
            tc.tile_pool(name="persist", bufs=1) as pp,
            tc.tile_pool(name="dram", bufs=1, space="DRAM") as dp,
        ):
            pk_sb = pp.tile([128, PCW], f32, tag="packf")
            pb_sb = pp.tile([128, PBW], bf16, tag="packb")
            w1a = pp.tile([128, NMA * 2 * HSH], f8, tag="w1a")
            v1 = pp.tile([128, IC * KN * T], f8, tag="v1")
            O_all = pp.tile([128, NQ * 4096], f8, tag="oall")
            ab_sb = pp.tile([128, 6], f32, tag="absb")

            bq_sb = pk_sb[:, PC_BQ:PC_BQ + 1]
            bk_sb = pk_sb[:, PC_BK:PC_BK + 1]
            bv_sb = pk_sb[:, PC_BV:PC_BV + 1]
            mask_sb = pk_sb[:, PC_MASK:PC_MASK + 2]
            b2_sb = pk_sb[0:KN, PC_B2:PC_B2 + 1]
            ones128 = pk_sb[:, PC_ONES:PC_ONES + 1]      # value 1/128
            sel_sb = pk_sb[0:HL, PC_SEL:PC_SEL + 128]
            bc1_sb = pk_sb[0:1, PC_BC1:PC_BC1 + 128]
            bnp_sb = pk_sb[0:HL, PC_BNP:PC_BNP + 8]
            bnp1_sb = pk_sb[0:1, PC_BNP1:PC_BNP1 + 4]
            eye8_sb = pk_sb[0:8, PC_EYE8:PC_EYE8 + 8]
            wb_sb = pk_sb[0:2, PC_WB:PC_WB + 5 * 128]
            magic_sb = pk_sb[0:2, PC_MAGIC:PC_MAGIC + 1]
            eye_sb = pb_sb[:, 0:128]
            w2_sb = [pb_sb[:, PB_W2 + j * KN:PB_W2 + (j + 1) * KN]
                     for j in range(5)]
            ind_sb = [pb_sb[0:8, PB_IND + h * T:PB_IND + (h + 1) * T]
                      for h in range(HL)]

            nc.sync.dma_start(pk_sb[:], pk_d)
            nc.scalar.dma_start(pb_sb[:], pb_d)
            w1av = w1a[:].rearrange("p (m x) -> p m x", m=NMA)
            for ch in range(3):
                nc.scalar.dma_start(
                    w1av[:, 16 * ch:16 * (ch + 1), :],
                    w1a_d[16 * ch:16 * (ch + 1)].transpose([1, 0, 2]),
                )
            # prefetch the exp act table set
            dummy = pp.tile([1, 1], f32, tag="dummy")
            nc.scalar.activation(dummy[:], ones128[0:1, :], AF.Exp,
                                 bias=0.0, scale=1.0)

            def rsqrt_dve(pool, dst, srcap, n, tag):
                u = pool.tile([n, 1], f32, tag=f"rqu{tag}")
                nc.vector.tensor_scalar(
                    u[:].bitcast(i32), srcap.bitcast(i32), 1, None,
                    op0=OP.logical_shift_right,
                )
                yb = pool.tile([n, 1], f32, tag=f"rqy{tag}")
                nc.vector.tensor_tensor(
                    yb[:].bitcast(i32), magic_sb[0:n, :].bitcast(i32),
                    u[:].bitcast(i32), op=OP.subtract,
                )
                h = pool.tile([n, 1], f32, tag=f"rqh{tag}")
                nc.vector.tensor_scalar_mul(h[:], srcap, 0.5)
                for it in range(2):
                    t3 = pool.tile([n, 1], f32, tag=f"rq3{tag}")
                    nc.vector.tensor_tensor(t3[:], yb[:], yb[:],
                                            op=OP.mult)
                    nc.vector.tensor_tensor(t3[:], h[:], t3[:],
                                            op=OP.mult)
                    nc.vector.tensor_scalar(t3[:], t3[:], -1.0, 1.5,
                                            op0=OP.mult, op1=OP.add)
                    nc.vector.tensor_tensor(yb[:], t3[:], yb[:],
                                            op=OP.mult)
                nc.vector.tensor_copy(dst, yb[:])

            # startup dummy collective to absorb cross-core launch skew
            zz = pp.tile([1, 1], f32, tag="zz")
            nc.vector.memset(zz[:], 0.0)
            zzd = dp.tile([1, 1], f32, tag="zzd")
            nc.sync.dma_start(zzd[:], zz[:])
            zzo = dp.tile([N_CORES, 1, 1], f32, tag="zzo",
                          addr_space="Shared")
            nc.gpsimd.collective_compute(
                "AllGather", OP.bypass, replica_groups=RG,
                ins=[zzd[:].opt()], outs=[zzo[:].opt()],
            )

            # attention-output layout (blocks of (ic,k,hl,b16) per half):
            # col = h*4096 + ic*2048 + k*32 + hl*16 + b16
            Ov = O_all[:].rearrange("p (q a k l b) -> p q a k l b",
                                    q=NQ, a=IC, k=KN, l=HL)
            # gathered layout: col = (h*8+c)*4096 + ic*2048 + k*32
            #                        + hl*16 + b16
            v1r = v1[:].rearrange("p (b a k r) -> p b a k r",
                                  b=16, a=IC, k=KN)

            fgls = []
            with tc.tile_pool(name="proj", bufs=1) as prp:
                qp = prp.tile([ROWS, BS * SL], bf16, tag="qp")
                kp = prp.tile([ROWS, BS * SL], bf16, tag="kp")
                vp = prp.tile([ROWS, BS * SL], bf16, tag="vp")

                # ---------- Phase A: qp/kp/vp = W[R,:] @ x + b ----------
                with (
                    tc.tile_pool(name="xin", bufs=4) as xp,
                    tc.tile_pool(name="wts", bufs=1) as wp,
                    tc.tile_pool(name="psA", bufs=4, space="PSUM") as psA,
                    tc.tile_pool(name="stat", bufs=1) as st,
                ):
                    ws = []
                    for ti, w_d in enumerate((wq_d, wk_d, wv_d)):
                        w_sb = wp.tile([FN, ROWS], bf16, tag=f"w{ti}",
                                       name=f"w{ti}")
                        nc.sync.dma_start(w_sb[:], w_d)
                        ws.append(w_sb)
                    bnsts = [
                        st.tile([ROWS, 16 * 6], f32, tag=f"bnst{ti}",
                                name=f"bnst{ti}")
                        for ti in range(3)
                    ]
                    for ti, (x_d, b_sb, dst) in enumerate(
                        ((q_d, bq_sb, qp), (k_d, bk_sb, kp),
                         (v_d, bv_sb, vp))
                    ):
                        for xc in range(4):
                            xcs = slice(xc * 2048, (xc + 1) * 2048)
                            x_sb = xp.tile([FN, 2048], bf16, tag="xch",
                                           name=f"x{ti}_{xc}")
                            nc.sync.dma_start(x_sb[:], x_d[:, xcs])
                            for n in range(4):
                                cs = slice(xc * 2048 + n * 512,
                                           xc * 2048 + (n + 1) * 512)
                                ncs = 4 * xc + n
                                ps = psA.tile([ROWS, 512], f32, tag="proj",
                                              name=f"proj{ti}_{ncs}")
                                nc.tensor.matmul(
                                    ps[:], ws[ti][:],
                                    x_sb[:, n * 512:(n + 1) * 512])
                                nc.scalar.activation(
                                    dst[:, cs], ps[:], AF.Identity,
                                    bias=b_sb, scale=1.0,
                                )
                                nc.vector.bn_stats(
                                    bnsts[ti][:, 6 * ncs:6 * (ncs + 1)],
                                    dst[:, cs],
                                )

                    # ---------- Phase B: per-head BN affine for q/k/v ----
                    with tc.tile_pool(name="psB", bufs=1,
                                      space="PSUM") as psB:
                        AB = st.tile([HL, 6], f32, tag="AB")
                        for ti in range(3):
                            gc, bc_ = 2 * ti, 2 * ti + 1
                            mv = st.tile([ROWS, 2], f32, tag=f"mv{ti}",
                                         name=f"mv{ti}")
                            nc.vector.bn_aggr(
                                mv[:],
                                bnsts[ti][:].rearrange(
                                    "p (c s) -> p c s", s=6
                                ),
                            )
                            stat2 = st.tile([ROWS, 2], f32, tag=f"s2{ti}",
                                            name=f"s2{ti}")
                            nc.vector.tensor_copy(stat2[:, 0:1], mv[:, 0:1])
                            nc.vector.scalar_tensor_tensor(
                                stat2[:, 1:2], mv[:, 0:1], mv[:, 0:1],
                                mv[:, 1:2], op0=OP.mult, op1=OP.add,
                            )
                            hs = psB.tile([HL, 2], f32, tag=f"hs{ti}",
                                          name=f"hs{ti}")
                            nc.tensor.matmul(hs[:], mask_sb, stat2[:])
                            mean_h = st.tile([HL, 1], f32, tag=f"mh{ti}",
                                             name=f"mh{ti}")
                            nc.vector.tensor_copy(mean_h[:], hs[:, 0:1])
                            tmp = st.tile([HL, 1], f32, tag=f"tp{ti}",
                                          name=f"tp{ti}")
                            nc.vector.tensor_tensor(
                                tmp[:], mean_h[:], mean_h[:], op=OP.mult
                            )
                            var_h = st.tile([HL, 1], f32, tag=f"vh{ti}",
                                            name=f"vh{ti}")
                            nc.vector.tensor_tensor(
                                var_h[:], hs[:, 1:2], tmp[:],
                                op=OP.subtract,
                            )
                            nc.vector.tensor_scalar_add(
                                var_h[:], var_h[:], EPS
                            )
                            rsq = st.tile([HL, 1], f32, tag=f"rq{ti}",
                                          name=f"rq{ti}")
                            rsqrt_dve(st, rsq[:], var_h[:], HL, f"b{ti}")
                            a_h = st.tile([HL, 1], f32, tag=f"ah{ti}",
                                          name=f"ah{ti}")
                            nc.vector.tensor_tensor(
                                a_h[:], bnp_sb[:, gc:gc + 1], rsq[:],
                                op=OP.mult,
                            )
                            tmp2 = st.tile([HL, 1], f32, tag=f"t2{ti}",
                                           name=f"t2{ti}")
                            nc.vector.tensor_tensor(
                                tmp2[:], mean_h[:], a_h[:], op=OP.mult
                            )
                            nc.vector.tensor_tensor(
                                AB[:, bc_:bc_ + 1], bnp_sb[:, bc_:bc_ + 1],
                                tmp2[:], op=OP.subtract,
                            )
                            nc.vector.tensor_copy(AB[:, gc:gc + 1], a_h[:])
                        bc_ps = psB.tile([128, 6], f32, tag="bcps")
                        nc.tensor.matmul(bc_ps[:], sel_sb, AB[:])
                        nc.vector.tensor_copy(ab_sb[:], bc_ps[:])

                # ---------- Phase C: attention, 2 heads, AG quarters ----
                with (
                    tc.tile_pool(name="stage", bufs=3) as sg,
                    tc.tile_pool(name="expp", bufs=2) as epool,
                    tc.tile_pool(name="vwp", bufs=3) as vwp,
                    tc.tile_pool(name="small", bufs=4) as smp,
                    tc.tile_pool(name="ps_sc", bufs=2, space="PSUM") as pssc,
                    tc.tile_pool(name="ps_vt", bufs=2, space="PSUM") as psvt,
                    tc.tile_pool(name="ps_uo", bufs=2, space="PSUM") as psuo,
                ):
                    for b in range(BS):
                        qq, b8 = divmod(b, QB)
                        bsl = slice(b * SL, (b + 1) * SL)
                        qw2 = sg.tile([128, SL], bf16, tag="qw")
                        nc.gpsimd.tensor_scalar(
                            qw2[:], qp[:, bsl], ab_sb[:, 0:1], ab_sb[:, 1:2],
                            op0=OP.mult, op1=OP.add,
                        )
                        kw2 = sg.tile([128, SL], bf16, tag="kw")
                        nc.gpsimd.tensor_scalar(
                            kw2[:], kp[:, bsl], ab_sb[:, 2:3], ab_sb[:, 3:4],
                            op0=OP.mult, op1=OP.add,
                        )
                        vw2 = sg.tile([128, SL], bf16, tag="vw")
                        nc.vector.tensor_scalar(
                            vw2[:], vp[:, bsl], ab_sb[:, 4:5], ab_sb[:, 5:6],
                            op0=OP.mult, op1=OP.add,
                        )
                        # scores both heads: [128(j in jc), hl*512 + i]
                        sc_ps = pssc.tile([128, 1024], f32, tag="scps")
                        for hl in range(HL):
                            r = slice(KN * hl, KN * (hl + 1))
                            for jc in range(2):
                                nc.tensor.matmul(
                                    sc_ps[:, hl * 512 + jc * 256:
                                          hl * 512 + (jc + 1) * 256],
                                    kw2[r, jc * 128:(jc + 1) * 128],
                                    qw2[r, :],
                                )
                        eT = epool.tile([128, 1024], bf16, tag="expT")
                        nc.scalar.activation(
                            eT[:], sc_ps[:], AF.Exp, bias=0.0, scale=0.125,
                        )
                        # vw transposed: [128(s in jc), k both heads]
                        vt_ps = psvt.tile([128, 256], bf16, tag="vtps")
                        for jc in range(2):
                            nc.tensor.transpose(
                                vt_ps[:, jc * 128:(jc + 1) * 128],
                                vw2[:, jc * 128:(jc + 1) * 128],
                                eye_sb,
                            )
                        vws2 = vwp.tile([128, 2 * 2 * (KN + 1)], bf16,
                                        tag="vws")
                        vws2v = vws2[:].rearrange(
                            "p (a h e) -> p a h e", a=2, h=2
                        )
                        for jc in range(2):
                            nc.vector.tensor_copy(
                                vws2v[:, jc, :, 0:KN],
                                vt_ps[:, jc * 128:(jc + 1) * 128].rearrange(
                                    "p (h e) -> p h e", h=2
                                ),
                            )
                        nc.vector.memset(vws2v[:, :, :, KN:KN + 1],
                                         1.0 / SCALE_O)
                        # unnormalized o + scaled exp row sums (col KN)
                        uo = psuo.tile([128, 2 * 2 * (KN + 1)], f32,
                                       tag="uo")
                        for hl in range(HL):
                            for ic in range(IC):
                                c0 = hl * 130 + ic * 65
                                for jc in range(2):
                                    nc.tensor.matmul(
                                        uo[:, c0:c0 + KN + 1],
                                        eT[:, hl * 512 + jc * 256 + ic * 128:
                                           hl * 512 + jc * 256 +
                                           (ic + 1) * 128],
                                        vws2v[:, jc, hl, :],
                                        start=(jc == 0), stop=(jc == 1),
                                    )
                        rec = smp.tile([128, 4], f32, tag="rec")
                        nc.vector.reciprocal(
                            rec[:].rearrange("p (h i e) -> p h i e",
                                             h=2, i=2),
                            uo[:].rearrange("p (h i e) -> p h i e",
                                            h=2, i=2)[:, :, :, KN:KN + 1],
                        )
                        for hl in range(HL):
                            for ic in range(IC):
                                c0 = hl * 130 + ic * 65
                                dst = Ov[:, qq, ic, :, hl, b8]
                                rc = rec[:, 2 * hl + ic:2 * hl + ic + 1]
                                if ic == 0 and hl == 0:
                                    nc.scalar.activation(
                                        dst, uo[:, c0:c0 + KN], AF.Identity,
                                        bias=0.0, scale=rc,
                                    )
                                else:
                                    nc.vector.tensor_scalar(
                                        dst, uo[:, c0:c0 + KN], rc, None,
                                        op0=OP.mult,
                                    )
                        if b8 == QB - 1:
                            floc = dp.tile([128, 4096], f8,
                                           tag=f"floc{qq}",
                                           name=f"floc{qq}")
                            nc.sync.dma_start(
                                floc[:],
                                O_all[:, qq * 4096:(qq + 1) * 4096],
                            )
                            fgl = dp.tile([N_CORES, 128, 4096], f8,
                                          tag=f"fgl{qq}", name=f"fgl{qq}",
                                          addr_space="Shared")
                            nc.gpsimd.collective_compute(
                                "AllGather", OP.bypass, replica_groups=RG,
                                ins=[floc[:].opt()], outs=[fgl[:].opt()],
                            )
                            fgls.append(fgl)

            for qq in range(NQ):
                for c in range(N_CORES):
                    blk = (qq * 8 + c) * 4096
                    nc.sync.dma_start(
                        v1[:, blk:blk + 4096], fgls[qq][c][:, :],
                    )

            # ---------- Phase F: fp8 DoubleRow para_linear1 ----------
            with (
                tc.tile_pool(name="pf", bufs=1) as fp,
                tc.tile_pool(name="scrp", bufs=2) as scp,
                tc.tile_pool(name="st1", bufs=1) as st1,
                tc.tile_pool(name="psH", bufs=1, space="PSUM") as psH,
                tc.tile_pool(name="psD", bufs=2, space="PSUM") as psD,
            ):
                w1b = fp.tile([128, NMB * 2 * HSH], f8, tag="w1b")
                w1bv = w1b[:].rearrange("p (m x) -> p m x", m=NMB)
                nc.scalar.dma_start(w1bv[:], w1b_d.transpose([1, 0, 2]))
                w1am = w1a[:].rearrange("p (m t j) -> p m t j",
                                        m=NMA, t=2)
                w1bm = w1b[:].rearrange("p (m t j) -> p m t j",
                                        m=NMB, t=2)
                h1ps = [
                    psH.tile([128, T], f32, tag=f"h1_{j}", name=f"h1ps{j}")
                    for j in range(5)
                ]
                for m in range(NM):
                    lt = w1am[:, m] if m < NMA else w1bm[:, m - NMA]
                    rhs = v1r[:, :, :, m, :].transpose([0, 2, 1, 3])
                    for j in range(5):
                        nc.tensor.matmul(
                            h1ps[j][:], lt[:, :, j * 128:(j + 1) * 128],
                            rhs, start=(m == 0), stop=False,
                            perf_mode=DR, skip_group_check=True,
                        )

                # ---------- BN1 stats (local heads) ----------
                Os = O_all[:].rearrange("p (x l b) -> p x l b",
                                        l=HL, b=16)
                st2 = st1.tile([128, 4], f32, tag="st2")
                for hl in range(HL):
                    npe = NQ * IC * KN * QB      # 4096 elems/partition
                    scrap = scp.tile([128, npe], bf16, tag="scrap",
                                     name=f"scrap{hl}")
                    sum1 = st1.tile([128, 1], f32, tag=f"sum{hl}")
                    nc.vector.tensor_scalar(
                        scrap[:].rearrange("p (x b) -> p x b", b=16),
                        Os[:, :, hl, :], 1.0, None, op0=OP.mult,
                        op1=OP.add, accum_out=sum1[:],
                    )
                    scrap2 = scp.tile([128, npe], bf16, tag="scrap",
                                      name=f"scrap2{hl}")
                    sq1 = st1.tile([128, 1], f32, tag=f"sq{hl}")
                    nc.scalar.activation(
                        scrap2[:].rearrange("p (x b) -> p x b", b=16),
                        Os[:, :, hl, :], AF.Square, accum_out=sq1[:],
                    )
                    nc.vector.tensor_scalar_mul(
                        st2[:, 2 * hl:2 * hl + 1], sum1[:], 1.0 / npe
                    )
                    nc.vector.tensor_scalar_mul(
                        st2[:, 2 * hl + 1:2 * hl + 2], sq1[:], 1.0 / npe
                    )
                hs1 = psD.tile([1, 4], f32, tag="psd", name="hs1")
                nc.tensor.matmul(hs1[:], ones128, st2[:])
                hsb = st1.tile([1, 4], f32, tag="hsb")
                nc.vector.tensor_copy(hsb[:], hs1[:])
                # local affine params -> [A_hl0, B_hl0, A_hl1, B_hl1,
                #                         a1'_hl0, a1'_hl1]
                arst = st1.tile([1, 6], f32, tag="arst")
                for hl in range(HL):
                    ms = hsb[:, 2 * hl:2 * hl + 1]       # 32*mean
                    qs = hsb[:, 2 * hl + 1:2 * hl + 2]   # 1024*E[x^2]
                    m2 = st1.tile([1, 1], f32, tag=f"m2_{hl}")
                    nc.vector.tensor_tensor(m2[:], ms, ms, op=OP.mult)
                    v32 = st1.tile([1, 1], f32, tag=f"v32_{hl}")
                    nc.vector.tensor_tensor(v32[:], qs, m2[:],
                                            op=OP.subtract)
                    varp = st1.tile([1, 1], f32, tag=f"vp_{hl}")
                    nc.vector.tensor_scalar(
                        varp[:], v32[:], 1.0 / (SCALE_O * SCALE_O), EPS,
                        op0=OP.mult, op1=OP.add,
                    )
                    rs = st1.tile([1, 1], f32, tag=f"rs_{hl}")
                    rsqrt_dve(st1, rs[:], varp[:], 1, f"g{hl}")
                    a1 = st1.tile([1, 1], f32, tag=f"a1_{hl}")
                    nc.vector.tensor_tensor(
                        a1[:], bnp1_sb[:, 2 * hl:2 * hl + 1], rs[:],
                        op=OP.mult,
                    )
                    inva = st1.tile([1, 1], f32, tag=f"ia_{hl}")
                    nc.vector.reciprocal(inva[:], a1[:])
                    mm = st1.tile([1, 1], f32, tag=f"mm_{hl}")
                    nc.vector.tensor_scalar_mul(mm[:], ms, 1.0 / SCALE_O)
                    am = st1.tile([1, 1], f32, tag=f"am_{hl}")
                    nc.vector.tensor_tensor(am[:], a1[:], mm[:],
                                            op=OP.mult)
                    c1 = st1.tile([1, 1], f32, tag=f"c1_{hl}")
                    nc.vector.tensor_tensor(
                        c1[:], bnp1_sb[:, 2 * hl + 1:2 * hl + 2], am[:],
                        op=OP.subtract,
                    )
                    ci = st1.tile([1, 1], f32, tag=f"ci_{hl}")
                    nc.vector.tensor_tensor(ci[:], c1[:], inva[:],
                                            op=OP.mult)
                    nc.vector.tensor_scalar_mul(
                        arst[:, 2 * hl:2 * hl + 1], ci[:],
                        SCALE_W * SCALE_O,
                    )
                    nc.vector.tensor_scalar_mul(
                        arst[:, 2 * hl + 1:2 * hl + 2], inva[:],
                        SCALE_W * SCALE_O,
                    )
                    nc.vector.tensor_scalar_mul(
                        arst[:, 4 + hl:5 + hl], a1[:], INV_SCALES,
                    )
                arst_d = dp.tile([1, 6], f32, tag="arstd")
                nc.sync.dma_start(arst_d[:], arst[:])
                absh = dp.tile([N_CORES, 1, 6], f32, tag="absh",
                               addr_space="Shared")
                nc.gpsimd.collective_compute(
                    "AllGather", OP.bypass, replica_groups=RG,
                    ins=[arst_d[:].opt()], outs=[absh[:].opt()],
                )
                ab8 = st1.tile([N_CORES, 6], f32, tag="ab8")
                nc.sync.dma_start(ab8[:], absh[:, 0, :])
                # transpose per-head params to rows
                abT = []
                for s in range(2):
                    tp = psD.tile([2, N_CORES], f32, tag="psd",
                                  name=f"abTp{s}")
                    nc.tensor.transpose(tp[:], ab8[:, 2 * s:2 * s + 2],
                                        eye8_sb)
                    sb = st1.tile([2, N_CORES], f32, tag=f"abT{s}")
                    nc.vector.tensor_copy(sb[:], tp[:])
                    abT.append(sb)
                a1s = []
                for hl in range(HL):
                    tp = psD.tile([1, N_CORES], f32, tag="psd",
                                  name=f"a1Tp{hl}")
                    nc.tensor.transpose(tp[:], ab8[:, 4 + hl:5 + hl],
                                        eye8_sb)
                    sb = st1.tile([1, N_CORES], f32, tag=f"a1s{hl}")
                    nc.vector.tensor_copy(sb[:], tp[:])
                    a1s.append(sb)
                # D rows: dts[(j,hl)] = [8c, 128p] = A_hl[c]*w1s + B_hl[c]*b1
                dts = st1.tile([8, 10 * 128], bf16, tag="dts")
                for j in range(5):
                    for hl in range(HL):
                        dtp = psD.tile([8, 128], f32, tag="psd",
                                       name=f"dtp{j}_{hl}")
                        nc.tensor.matmul(
                            dtp[:], abT[hl][:],
                            wb_sb[:, j * 128:(j + 1) * 128],
                        )
                        nc.vector.tensor_copy(
                            dts[:, (j * 2 + hl) * 128:
                                (j * 2 + hl + 1) * 128],
                            dtp[:],
                        )
                # a1 broadcast [64, 16] (col = hl*8 + c)
                a1bc = st1.tile([KN, 16], f32, tag="a1bc")
                for hl in range(HL):
                    a1p = psD.tile([KN, N_CORES], f32, tag="psd",
                                   name=f"a1p{hl}")
                    nc.tensor.matmul(a1p[:], bc1_sb[:, 0:KN], a1s[hl][:])
                    nc.vector.tensor_copy(
                        a1bc[:, hl * 8:(hl + 1) * 8], a1p[:]
                    )
                # D-add into h1 psums
                for j in range(5):
                    for hl in range(HL):
                        nc.tensor.matmul(
                            h1ps[j][:],
                            dts[:, (j * 2 + hl) * 128:
                                (j * 2 + hl + 1) * 128],
                            ind_sb[hl],
                            start=False, stop=(hl == HL - 1),
                            skip_group_check=True,
                        )

                # ---------- leaky + W2 + scale + AllReduce + sigmoid ----
                h1sb = [
                    fp.tile([128, T], bf16, tag=f"h1s_{j}", name=f"h1s{j}")
                    for j in range(5)
                ]
                h1af = [
                    fp.tile([128, T], bf16, tag=f"h1a_{j}", name=f"h1a{j}")
                    for j in range(5)
                ]
                for j in range(5):
                    nc.scalar.activation(
                        h1af[j][:], h1ps[j][:], AF.Identity,
                        bias=0.0, scale=1.0,
                    )
                    nc.vector.scalar_tensor_tensor(
                        h1sb[j][:], h1af[j][:], SLOPE, h1af[j][:],
                        op0=OP.mult, op1=OP.max,
                    )
                ps2 = psH.tile([KN, T], f32, tag="out2")
                for j in range(5):
                    nc.tensor.matmul(
                        ps2[:], w2_sb[j], h1sb[j][:],
                        start=(j == 0), stop=(j == 4),
                    )
                o2f = fp.tile([KN, T], f32, tag="o2f")
                nc.vector.tensor_copy(o2f[:], ps2[:])
                o2s = fp.tile([KN, T], f32, tag="o2s")
                for h in range(NQ):
                    for c in range(N_CORES):
                        for hl in range(HL):
                            g = h * 256 + c * 32 + hl * 16
                            nc.vector.tensor_scalar(
                                o2s[:, g:g + 16], o2f[:, g:g + 16],
                                a1bc[:, hl * 8 + c:hl * 8 + c + 1], None,
                                op0=OP.mult,
                            )
                arin = dp.tile([KN, T], f32, tag="arin")
                nc.sync.dma_start(arin[:], o2s[:])
                arout = dp.tile([KN, T], f32, tag="arout",
                                addr_space="Shared")
                nc.gpsimd.collective_compute(
                    "AllReduce", OP.add, replica_groups=RG,
                    ins=[arin[:].opt()], outs=[arout[:].opt()],
                )
                arsb = fp.tile([KN, T], f32, tag="arsb")
                nc.sync.dma_start(arsb[:], arout[:])
                fin = fp.tile([KN, T], f32, tag="fin")
                nc.scalar.activation(
                    fin[:], arsb[:], AF.Sigmoid, bias=b2_sb, scale=1.0
                )
                nc.sync.dma_start(out_d, fin[:])

    nc.compile()
    return nc


def _dup_wT(W, c):
    W = np.asarray(W, np.float32)
    cols = [W[8 * c + ST * hl: 8 * c + ST * hl + KN, :].T for hl in range(HL)]
    return np.concatenate(cols, axis=1)


def _dup_b(b, c):
    b = np.asarray(b, np.float32)
    rows = [b[8 * c + ST * hl: 8 * c + ST * hl + KN] for hl in range(HL)]
    return np.ascontiguousarray(np.concatenate(rows))


def _prep_in_maps(inputs):
    import ml_dtypes

    f = np.float32
    bf = ml_dtypes.bfloat16
    f8 = ml_dtypes.float8_e4m3
    q = np.asarray(inputs["q"], f)
    k = np.asarray(inputs["k"], f)
    v = np.asarray(inputs["v"], f)
    qh = np.ascontiguousarray(
        q[:, 0].transpose(1, 0, 2).reshape(FN, BS * SL).astype(bf))
    kh = np.ascontiguousarray(
        k[:, 0].transpose(1, 0, 2).reshape(FN, BS * SL).astype(bf))
    vh = np.ascontiguousarray(
        v[:, 0].transpose(1, 0, 2).reshape(FN, BS * SL).astype(bf))
    W1 = np.asarray(inputs["W1"], f)
    W1p = np.zeros((HIDP, SL * KN), f)
    W1p[:HID] = W1
    # device contraction row ((ic*64+kk)*128+p) = orig col ((ic*128+p)*64+kk)
    W1r = W1p.reshape(HIDP, IC, 128, KN).transpose(1, 3, 2, 0).reshape(
        SL * KN, HIDP)
    # DoubleRow pairs: pair axis = ic -> [m=kk, 128 p, 2 (ic), HIDP]
    W1m = W1r.reshape(IC, KN, 128, HIDP).transpose(1, 2, 0, 3)
    w1sum = np.zeros((HIDP,), f)
    w1sum[:HID] = W1.sum(axis=1)
    b1p = np.zeros((HIDP,), f)
    b1p[:HID] = np.asarray(inputs["b1"], f)
    W2T = np.zeros((HIDP, KN), f)
    W2T[:HID] = np.asarray(inputs["W2"], f).T
    b2 = np.asarray(inputs["b2"], f)
    # token order: t = h*256 + c*32 + hl*16 + b16; head = c*2 + hl
    tt = np.arange(T)
    thead = 2 * ((tt % 256) // 32) + (tt % 32) // 16
    in_maps = []
    for c in range(N_CORES):
        h0 = HL * c
        packf = np.zeros((128, PCW), f)
        packf[:, PC_BQ] = _dup_b(inputs["bq"], c)
        packf[:, PC_BK] = _dup_b(inputs["bk"], c)
        packf[:, PC_BV] = _dup_b(inputs["bv"], c)
        for hl in range(HL):
            packf[KN * hl:KN * (hl + 1), PC_MASK + hl] = 1.0 / KN
            packf[hl, PC_SEL + hl * KN:PC_SEL + (hl + 1) * KN] = 1.0
        packf[0:KN, PC_B2] = b2
        packf[:, PC_ONES] = 1.0 / 128.0
        packf[0, PC_BC1:PC_BC1 + 128] = 1.0
        for hl in range(HL):
            packf[hl, PC_BNP:PC_BNP + 8] = [
                inputs["gq"][h0 + hl], inputs["beq"][h0 + hl],
                inputs["gk"][h0 + hl], inputs["bek"][h0 + hl],
                inputs["gv"][h0 + hl], inputs["bev"][h0 + hl],
                inputs["g1"][h0 + hl], inputs["be1"][h0 + hl],
            ]
        packf[0, PC_BNP1:PC_BNP1 + 4] = [
            inputs["g1"][h0], inputs["be1"][h0],
            inputs["g1"][h0 + 1], inputs["be1"][h0 + 1],
        ]
        packf[0:8, PC_EYE8:PC_EYE8 + 8] = np.eye(8, dtype=f)
        packf[0:2, PC_MAGIC] = np.frombuffer(
            np.array([0x5F3759DF, 0x5F3759DF], np.uint32).tobytes(),
            dtype=f)
        packf[0, PC_WB:PC_WB + 5 * 128] = w1sum[c * HSH:(c + 1) * HSH]
        packf[1, PC_WB:PC_WB + 5 * 128] = b1p[c * HSH:(c + 1) * HSH]
        packb = np.zeros((128, PBW), f)
        packb[:, 0:128] = np.eye(128, dtype=f)
        W2c = W2T[c * HSH:(c + 1) * HSH, :]
        for j in range(5):
            packb[:, PB_W2 + j * KN:PB_W2 + (j + 1) * KN] = \
                W2c[j * 128:(j + 1) * 128, :]
        for hl in range(HL):
            for cc in range(N_CORES):
                head = cc * 2 + hl
                packb[cc, PB_IND + hl * T:PB_IND + (hl + 1) * T] = \
                    (thead == head).astype(f)
        w1c = np.ascontiguousarray(
            (W1m[:, :, :, c * HSH:(c + 1) * HSH] * SCALE_W)
            .transpose(0, 1, 2, 3)
            .reshape(NM, 128, 2 * HSH).astype(f8))
        m = {
            "qh": qh, "kh": kh, "vh": vh,
            "wqT": np.ascontiguousarray(_dup_wT(inputs["Wq"], c).astype(bf)),
            "wkT": np.ascontiguousarray(_dup_wT(inputs["Wk"], c).astype(bf)),
            "wvT": np.ascontiguousarray(_dup_wT(inputs["Wv"], c).astype(bf)),
            "packf": packf,
            "packb": np.ascontiguousarray(packb.astype(bf)),
            "w1a": np.ascontiguousarray(w1c[:NMA]),
            "w1b": np.ascontiguousarray(w1c[NMA:]),
        }
        in_maps.append(m)
    return in_maps


def _unshard(o):
    # out cols: t = h*256 + c*32 + hl*16 + b16; head = c*2+hl, b = h*16+b16
    out = (
        np.asarray(o, np.float32)
        .reshape(KN, NQ, N_CORES, HL, QB)
        .transpose(1, 4, 2, 3, 0)
        .reshape(BS, HEADS, KN)[:, None]
    )
    return np.ascontiguousarray(out.astype(np.float32))


def kernel(**inputs):
    global _prog
    if _prog is None:
        _prog = _build()
    from concourse.bass_utils import run_bass_kernel_spmd

    in_maps = _prep_in_maps(inputs)
    res = run_bass_kernel_spmd(_prog, in_maps, list(range(N_CORES)))
    return _unshard(res.results[0]["out"])


# revision 10
# speedup vs baseline: 3.4936x; 1.0345x over previous
"""Trainium2 Bass kernel for nn_MultiHeadAttention_34144990003301 (v5).

Head-parallel attention (2 heads/core), bf16 q/k/v datapath.
BatchNorm1 is POSTPONED past para_linear1: the affine commutes through
the linear layer (h1 = a1*(W1@O) + c1*rowsum(W1) + b1), so raw attention
output O is quantized to fp8e4 (scaled x32), AllGathered in quarters of
the batch DURING attention, and para_linear1 runs as fp8 DoubleRow
matmuls (2 contraction tiles per MM, W1 scaled x2^14). The BN1 bias term
enters as a tiny rank-2 matmul (D) appended to the accumulation; the
per-head scale a1 is applied after W2 (leaky(a*x) = a*leaky(x), a>0).
BN1 stats are AllGathered as 6 floats/core. One AllReduce of the W2
partials, sigmoid on device.

kernel(**inputs) takes the full unsharded inputs, returns [32,1,16,64] f32.
"""

import numpy as np

BS, HEADS, FN, SL, KN, ST = 32, 16, 124, 256, 64, 4
HID = 5000
HIDP = 5120                    # zero-padded hid
EPS = 1e-5
SLOPE = 0.01
N_CORES = 8
HL = HEADS // N_CORES          # 2 local heads per core
ROWS = HL * KN                 # 128 projected rows (per-head 64, duplicated)
T = BS * HEADS                 # 512 global tokens
HSH = HIDP // N_CORES          # 640 hid cols per core (5 blocks of 128)
IC = SL // 128                 # 2 i-chunks
NM = 64                        # DoubleRow kt-pairs (128 kt tiles / 2)
NMA = 48                       # pairs resident early (w1a)
NMB = NM - NMA                 # pairs streamed late (w1b)
NQ = 2                         # AllGather half-chunks
QB = BS // NQ                  # 16 batches per chunk
SCALE_W = 2.0 ** 14            # W1 fp8 scale
SCALE_O = 32.0                 # attention-output fp8 scale
INV_SCALES = 1.0 / (SCALE_W * SCALE_O)
# packed f32 const columns
PC_BQ, PC_BK, PC_BV = 0, 1, 2
PC_MASK = 3                    # 2 cols
PC_B2 = 5
PC_ONES = 6                    # value 1/128
PC_SEL = 7                     # 128 cols (rows 0:2)
PC_BC1 = 135                   # 128 cols (row 0) value 1.0
PC_BNP = 263                   # 8 cols (rows 0:2)
PC_BNP1 = 271                  # 4 cols (row 0)
PC_EYE8 = 275                  # 8 cols (rows 0:8)
PC_WB = 283                    # 640 cols (rows 0:2): w1s | b1 shard
PC_MAGIC = 923                 # 1 col rows 0:2: quake-rsqrt magic bits
PCW = 924
# packed bf16 cols: eye128 | w2 (5*KN) | ind0 (512) | ind1 (512)
PB_W2 = 128
PB_IND = 128 + 5 * KN
PBW = PB_IND + 2 * T

_prog = None


def _build():
    import concourse.bacc as bacc
    import concourse.tile as tile
    import concourse.mybir as mybir

    f32 = mybir.dt.float32
    bf16 = mybir.dt.bfloat16
    f8 = mybir.dt.float8e4
    AF = mybir.ActivationFunctionType
    OP = mybir.AluOpType
    DR = mybir.MatmulPerfMode.DoubleRow
    RG = [list(range(N_CORES))]

    i32 = mybir.dt.int32

    nc = bacc.Bacc("TRN2", target_bir_lowering=False, debug=False,
                   num_devices=N_CORES)

    def din(name, shape, dt=f32):
        return nc.dram_tensor(
            name, list(shape), dt, kind="ExternalInput"
        ).ap()

    q_d = din("qh", (FN, BS * SL), bf16)
    k_d = din("kh", (FN, BS * SL), bf16)
    v_d = din("vh", (FN, BS * SL), bf16)
    wq_d = din("wqT", (FN, ROWS), bf16)
    wk_d = din("wkT", (FN, ROWS), bf16)
    wv_d = din("wvT", (FN, ROWS), bf16)
    pk_d = din("packf", (128, PCW))
    pb_d = din("packb", (128, PBW), bf16)
    w1a_d = din("w1a", (NMA, 128, 2 * HSH), f8)
    w1b_d = din("w1b", (NMB, 128, 2 * HSH), f8)
    out_d = nc.dram_tensor("out", [KN, T], f32, kind="ExternalOutput").ap()

    with tile.TileContext(nc) as tc:
        with (
            tc.tile_pool(name="persist", bufs=1) as pp,
            tc.tile_pool(name="dram", bufs=1, space="DRAM") as dp,
        ):
            pk_sb = pp.tile([128, PCW], f32, tag="packf")
            pb_sb = pp.tile([128, PBW], bf16, tag="packb")
            w1a = pp.tile([128, NMA * 2 * HSH], f8, tag="w1a")
            v1 = pp.tile([128, IC * KN * T], f8, tag="v1")
            O_all = pp.tile([128, NQ * 4096], f8, tag="oall")
            ab_sb = pp.tile([128, 6], f32, tag="absb")

            bq_sb = pk_sb[:, PC_BQ:PC_BQ + 1]
            bk_sb = pk_sb[:, PC_BK:PC_BK + 1]
            bv_sb = pk_sb[:, PC_BV:PC_BV + 1]
            mask_sb = pk_sb[:, PC_MASK:PC_MASK + 2]
            b2_sb = pk_sb[0:KN, PC_B2:PC_B2 + 1]
            ones128 = pk_sb[:, PC_ONES:PC_ONES + 1]      # value 1/128
            sel_sb = pk_sb[0:HL, PC_SEL:PC_SEL + 128]
            bc1_sb = pk_sb[0:1, PC_BC1:PC_BC1 + 128]
            bnp_sb = pk_sb[0:HL, PC_BNP:PC_BNP + 8]
            bnp1_sb = pk_sb[0:1, PC_BNP1:PC_BNP1 + 4]
            eye8_sb = pk_sb[0:8, PC_EYE8:PC_EYE8 + 8]
            wb_sb = pk_sb[0:2, PC_WB:PC_WB + 5 * 128]
            magic_sb = pk_sb[0:2, PC_MAGIC:PC_MAGIC + 1]
            eye_sb = pb_sb[:, 0:128]
            w2_sb = [pb_sb[:, PB_W2 + j * KN:PB_W2 + (j + 1) * KN]
                     for j in range(5)]
            ind_sb = [pb_sb[0:8, PB_IND + h * T:PB_IND + (h + 1) * T]
                      for h in range(HL)]

            nc.sync.dma_start(pk_sb[:], pk_d)
            nc.sync.dma_start(pb_sb[:], pb_d)
            w1av = w1a[:].rearrange("p (m x) -> p m x", m=NMA)
            # prefetch the exp act table set
            dummy = pp.tile([1, 1], f32, tag="dummy")
            nc.scalar.activation(dummy[:], ones128[0:1, :], AF.Exp,
                                 bias=0.0, scale=1.0)

            def rsqrt_dve(pool, dst, srcap, n, tag):
                u = pool.tile([n, 1], f32, tag=f"rqu{tag}")
                nc.vector.tensor_scalar(
                    u[:].bitcast(i32), srcap.bitcast(i32), 1, None,
                    op0=OP.logical_shift_right,
                )
                yb = pool.tile([n, 1], f32, tag=f"rqy{tag}")
                nc.vector.tensor_tensor(
                    yb[:].bitcast(i32), magic_sb[0:n, :].bitcast(i32),
                    u[:].bitcast(i32), op=OP.subtract,
                )
                h = pool.tile([n, 1], f32, tag=f"rqh{tag}")
                nc.vector.tensor_scalar_mul(h[:], srcap, 0.5)
                for it in range(2):
                    t3 = pool.tile([n, 1], f32, tag=f"rq3{tag}")
                    nc.vector.tensor_tensor(t3[:], yb[:], yb[:],
                                            op=OP.mult)
                    nc.vector.tensor_tensor(t3[:], h[:], t3[:],
                                            op=OP.mult)
                    nc.vector.tensor_scalar(t3[:], t3[:], -1.0, 1.5,
                                            op0=OP.mult, op1=OP.add)
                    nc.vector.tensor_tensor(yb[:], t3[:], yb[:],
                                            op=OP.mult)
                nc.vector.tensor_copy(dst, yb[:])

            # startup dummy collective to absorb cross-core launch skew
            zz = pp.tile([1, 1], f32, tag="zz")
            nc.vector.memset(zz[:], 0.0)
            zzd = dp.tile([1, 1], f32, tag="zzd")
            nc.sync.dma_start(zzd[:], zz[:])
            zzo = dp.tile([N_CORES, 1, 1], f32, tag="zzo",
                          addr_space="Shared")
            nc.gpsimd.collective_compute(
                "AllGather", OP.bypass, replica_groups=RG,
                ins=[zzd[:].opt()], outs=[zzo[:].opt()],
            )

            # attention-output layout (blocks of (ic,k,hl,b16) per half):
            # col = h*4096 + ic*2048 + k*32 + hl*16 + b16
            Ov = O_all[:].rearrange("p (q a k l b) -> p q a k l b",
                                    q=NQ, a=IC, k=KN, l=HL)
            # gathered layout: col = (h*8+c)*4096 + ic*2048 + k*32
            #                        + hl*16 + b16
            v1r = v1[:].rearrange("p (b a k r) -> p b a k r",
                                  b=16, a=IC, k=KN)

            fgls = []
            with tc.tile_pool(name="proj", bufs=1) as prp:
                qp = prp.tile([ROWS, BS * SL], bf16, tag="qp")
                kp = prp.tile([ROWS, BS * SL], bf16, tag="kp")
                vp = prp.tile([ROWS, BS * SL], bf16, tag="vp")

                # ---------- Phase A: qp/kp/vp = W[R,:] @ x + b ----------
                with (
                    tc.tile_pool(name="xin", bufs=4) as xp,
                    tc.tile_pool(name="wts", bufs=1) as wp,
                    tc.tile_pool(name="psA", bufs=4, space="PSUM") as psA,
                    tc.tile_pool(name="stat", bufs=1) as st,
                ):
                    ws = []
                    for ti, w_d in enumerate((wq_d, wk_d, wv_d)):
                        w_sb = wp.tile([FN, ROWS], bf16, tag=f"w{ti}",
                                       name=f"w{ti}")
                        nc.sync.dma_start(w_sb[:], w_d)
                        ws.append(w_sb)
                    bnsts = [
                        st.tile([ROWS, 16 * 6], f32, tag=f"bnst{ti}",
                                name=f"bnst{ti}")
                        for ti in range(3)
                    ]
                    for ti, (x_d, b_sb, dst) in enumerate(
                        ((q_d, bq_sb, qp), (k_d, bk_sb, kp),
                         (v_d, bv_sb, vp))
                    ):
                        for xc in range(4):
                            xcs = slice(xc * 2048, (xc + 1) * 2048)
                            x_sb = xp.tile([FN, 2048], bf16, tag="xch",
                                           name=f"x{ti}_{xc}")
                            nc.sync.dma_start(x_sb[:], x_d[:, xcs])
                            for n in range(4):
                                cs = slice(xc * 2048 + n * 512,
                                           xc * 2048 + (n + 1) * 512)
                                ncs = 4 * xc + n
                                ps = psA.tile([ROWS, 512], f32, tag="proj",
                                              name=f"proj{ti}_{ncs}")
                                nc.tensor.matmul(
                                    ps[:], ws[ti][:],
                                    x_sb[:, n * 512:(n + 1) * 512])
                                nc.scalar.activation(
                                    dst[:, cs], ps[:], AF.Identity,
                                    bias=b_sb, scale=1.0,
                                )
                                nc.vector.bn_stats(
                                    bnsts[ti][:, 6 * ncs:6 * (ncs + 1)],
                                    dst[:, cs],
                                )

                    # ---------- Phase B: per-head BN affine for q/k/v ----
                    with tc.tile_pool(name="psB", bufs=1,
                                      space="PSUM") as psB:
                        AB = st.tile([HL, 6], f32, tag="AB")
                        for ti in range(3):
                            gc, bc_ = 2 * ti, 2 * ti + 1
                            mv = st.tile([ROWS, 2], f32, tag=f"mv{ti}",
                                         name=f"mv{ti}")
                            nc.vector.bn_aggr(
                                mv[:],
                                bnsts[ti][:].rearrange(
                                    "p (c s) -> p c s", s=6
                                ),
                            )
                            stat2 = st.tile([ROWS, 2], f32, tag=f"s2{ti}",
                                            name=f"s2{ti}")
                            nc.vector.tensor_copy(stat2[:, 0:1], mv[:, 0:1])
                            nc.vector.scalar_tensor_tensor(
                                stat2[:, 1:2], mv[:, 0:1], mv[:, 0:1],
                                mv[:, 1:2], op0=OP.mult, op1=OP.add,
                            )
                            hs = psB.tile([HL, 2], f32, tag=f"hs{ti}",
                                          name=f"hs{ti}")
                            nc.tensor.matmul(hs[:], mask_sb, stat2[:])
                            mean_h = st.tile([HL, 1], f32, tag=f"mh{ti}",
                                             name=f"mh{ti}")
                            nc.vector.tensor_copy(mean_h[:], hs[:, 0:1])
                            tmp = st.tile([HL, 1], f32, tag=f"tp{ti}",
                                          name=f"tp{ti}")
                            nc.vector.tensor_tensor(
                                tmp[:], mean_h[:], mean_h[:], op=OP.mult
                            )
                            var_h = st.tile([HL, 1], f32, tag=f"vh{ti}",
                                            name=f"vh{ti}")
                            nc.vector.tensor_tensor(
                                var_h[:], hs[:, 1:2], tmp[:],
                                op=OP.subtract,
                            )
                            nc.vector.tensor_scalar_add(
                                var_h[:], var_h[:], EPS
                            )
                            rsq = st.tile([HL, 1], f32, tag=f"rq{ti}",
                                          name=f"rq{ti}")
                            rsqrt_dve(st, rsq[:], var_h[:], HL, f"b{ti}")
                            a_h = st.tile([HL, 1], f32, tag=f"ah{ti}",
                                          name=f"ah{ti}")
                            nc.vector.tensor_tensor(
                                a_h[:], bnp_sb[:, gc:gc + 1], rsq[:],
                                op=OP.mult,
                            )
                            tmp2 = st.tile([HL, 1], f32, tag=f"t2{ti}",
                                           name=f"t2{ti}")
                            nc.vector.tensor_tensor(
                                tmp2[:], mean_h[:], a_h[:], op=OP.mult
                            )
                            nc.vector.tensor_tensor(
                                AB[:, bc_:bc_ + 1], bnp_sb[:, bc_:bc_ + 1],
                                tmp2[:], op=OP.subtract,
                            )
                            nc.vector.tensor_copy(AB[:, gc:gc + 1], a_h[:])
                        bc_ps = psB.tile([128, 6], f32, tag="bcps")
                        nc.tensor.matmul(bc_ps[:], sel_sb, AB[:])
                        nc.vector.tensor_copy(ab_sb[:], bc_ps[:])

                # W1 stream on the sync queue (idle during attention)
                for ch in range(3):
                    nc.sync.dma_start(
                        w1av[:, 16 * ch:16 * (ch + 1), :],
                        w1a_d[16 * ch:16 * (ch + 1)].transpose([1, 0, 2]),
                    )

                # ---------- Phase C: attention, 2 heads, AG quarters ----
                with (
                    tc.tile_pool(name="stage", bufs=4) as sg,
                    tc.tile_pool(name="expp", bufs=3) as epool,
                    tc.tile_pool(name="vwp", bufs=4) as vwp,
                    tc.tile_pool(name="small", bufs=6) as smp,
                    tc.tile_pool(name="ps_sc", bufs=2, space="PSUM") as pssc,
                    tc.tile_pool(name="ps_vt", bufs=2, space="PSUM") as psvt,
                    tc.tile_pool(name="ps_uo", bufs=2, space="PSUM") as psuo,
                ):
                    for b in range(BS):
                        qq, b8 = divmod(b, QB)
                        bsl = slice(b * SL, (b + 1) * SL)
                        qw2 = sg.tile([128, SL], bf16, tag="qw")
                        nc.gpsimd.tensor_scalar(
                            qw2[:], qp[:, bsl], ab_sb[:, 0:1], ab_sb[:, 1:2],
                            op0=OP.mult, op1=OP.add,
                        )
                        kw2 = sg.tile([128, SL], bf16, tag="kw")
                        nc.gpsimd.tensor_scalar(
                            kw2[:], kp[:, bsl], ab_sb[:, 2:3], ab_sb[:, 3:4],
                            op0=OP.mult, op1=OP.add,
                        )
                        vw2 = sg.tile([128, SL], bf16, tag="vw")
                        nc.vector.tensor_scalar(
                            vw2[:], vp[:, bsl], ab_sb[:, 4:5], ab_sb[:, 5:6],
                            op0=OP.mult, op1=OP.add,
                        )
                        # scores both heads: [128(j in jc), hl*512 + i]
                        sc_ps = pssc.tile([128, 1024], f32, tag="scps")
                        for hl in range(HL):
                            r = slice(KN * hl, KN * (hl + 1))
                            for jc in range(2):
                                nc.tensor.matmul(
                                    sc_ps[:, hl * 512 + jc * 256:
                                          hl * 512 + (jc + 1) * 256],
                                    kw2[r, jc * 128:(jc + 1) * 128],
                                    qw2[r, :],
                                )
                        eT = epool.tile([128, 1024], bf16, tag="expT")
                        nc.scalar.activation(
                            eT[:], sc_ps[:], AF.Exp, bias=0.0, scale=0.125,
                        )
                        # vw transposed: [128(s in jc), k both heads]
                        vt_ps = psvt.tile([128, 256], bf16, tag="vtps")
                        for jc in range(2):
                            nc.tensor.transpose(
                                vt_ps[:, jc * 128:(jc + 1) * 128],
                                vw2[:, jc * 128:(jc + 1) * 128],
                                eye_sb,
                            )
                        vws2 = vwp.tile([128, 2 * 2 * (KN + 1)], bf16,
                                        tag="vws")
                        vws2v = vws2[:].rearrange(
                            "p (a h e) -> p a h e", a=2, h=2
                        )
                        for jc in range(2):
                            nc.vector.tensor_copy(
                                vws2v[:, jc, :, 0:KN],
                                vt_ps[:, jc * 128:(jc + 1) * 128].rearrange(
                                    "p (h e) -> p h e", h=2
                                ),
                            )
                        nc.vector.memset(vws2v[:, :, :, KN:KN + 1],
                                         1.0 / SCALE_O)
                        # unnormalized o + scaled exp row sums (col KN)
                        uo = psuo.tile([128, 2 * 2 * (KN + 1)], f32,
                                       tag="uo")
                        for hl in range(HL):
                            for ic in range(IC):
                                c0 = hl * 130 + ic * 65
                                for jc in range(2):
                                    nc.tensor.matmul(
                                        uo[:, c0:c0 + KN + 1],
                                        eT[:, hl * 512 + jc * 256 + ic * 128:
                                           hl * 512 + jc * 256 +
                                           (ic + 1) * 128],
                                        vws2v[:, jc, hl, :],
                                        start=(jc == 0), stop=(jc == 1),
                                    )
                        rec = smp.tile([128, 4], f32, tag="rec")
                        nc.vector.reciprocal(
                            rec[:].rearrange("p (h i e) -> p h i e",
                                             h=2, i=2),
                            uo[:].rearrange("p (h i e) -> p h i e",
                                            h=2, i=2)[:, :, :, KN:KN + 1],
                        )
                        for hl in range(HL):
                            for ic in range(IC):
                                c0 = hl * 130 + ic * 65
                                dst = Ov[:, qq, ic, :, hl, b8]
                                rc = rec[:, 2 * hl + ic:2 * hl + ic + 1]
                                if ic == 0 and hl == 0:
                                    nc.scalar.activation(
                                        dst, uo[:, c0:c0 + KN], AF.Identity,
                                        bias=0.0, scale=rc,
                                    )
                                else:
                                    nc.vector.tensor_scalar(
                                        dst, uo[:, c0:c0 + KN], rc, None,
                                        op0=OP.mult,
                                    )
                        if b8 == QB - 1:
                            floc = dp.tile([128, 4096], f8,
                                           tag=f"floc{qq}",
                                           name=f"floc{qq}")
                            nc.sync.dma_start(
                                floc[:],
                                O_all[:, qq * 4096:(qq + 1) * 4096],
                            )
                            fgl = dp.tile([N_CORES, 128, 4096], f8,
                                          tag=f"fgl{qq}", name=f"fgl{qq}",
                                          addr_space="Shared")
                            nc.gpsimd.collective_compute(
                                "AllGather", OP.bypass, replica_groups=RG,
                                ins=[floc[:].opt()], outs=[fgl[:].opt()],
                            )
                            fgls.append(fgl)
                            for c in range(N_CORES):
                                blk = (qq * 8 + c) * 4096
                                nc.sync.dma_start(
                                    v1[:, blk:blk + 4096],
                                    fgl[c][:, :],
                                )

            # ---------- Phase F: fp8 DoubleRow para_linear1 ----------
            with (
                tc.tile_pool(name="pf", bufs=1) as fp,
                tc.tile_pool(name="scrp", bufs=2) as scp,
                tc.tile_pool(name="st1", bufs=1) as st1,
                tc.tile_pool(name="psH", bufs=1, space="PSUM") as psH,
                tc.tile_pool(name="psD", bufs=2, space="PSUM") as psD,
            ):
                w1b = fp.tile([128, NMB * 2 * HSH], f8, tag="w1b")
                w1bv = w1b[:].rearrange("p (m x) -> p m x", m=NMB)
                nc.sync.dma_start(w1bv[:], w1b_d.transpose([1, 0, 2]))
                w1am = w1a[:].rearrange("p (m t j) -> p m t j",
                                        m=NMA, t=2)
                w1bm = w1b[:].rearrange("p (m t j) -> p m t j",
                                        m=NMB, t=2)
                h1ps = [
                    psH.tile([128, T], f32, tag=f"h1_{j}", name=f"h1ps{j}")
                    for j in range(5)
                ]
                for m in range(NM):
                    lt = w1am[:, m] if m < NMA else w1bm[:, m - NMA]
                    rhs = v1r[:, :, :, m, :].transpose([0, 2, 1, 3])
                    for j in range(5):
                        nc.tensor.matmul(
                            h1ps[j][:], lt[:, :, j * 128:(j + 1) * 128],
                            rhs, start=(m == 0), stop=False,
                            perf_mode=DR, skip_group_check=True,
                        )

                # ---------- BN1 stats (local heads) ----------
                Os = O_all[:].rearrange("p (x l b) -> p x l b",
                                        l=HL, b=16)
                st2 = st1.tile([128, 4], f32, tag="st2")
                for hl in range(HL):
                    npe = NQ * IC * KN * QB      # 4096 elems/partition
                    scrap = scp.tile([128, npe], bf16, tag="scrap",
                                     name=f"scrap{hl}")
                    sum1 = st1.tile([128, 1], f32, tag=f"sum{hl}")
                    nc.vector.tensor_scalar(
                        scrap[:].rearrange("p (x b) -> p x b", b=16),
                        Os[:, :, hl, :], 1.0, None, op0=OP.mult,
                        op1=OP.add, accum_out=sum1[:],
                    )
                    scrap2 = scp.tile([128, npe], bf16, tag="scrap",
                                      name=f"scrap2{hl}")
                    sq1 = st1.tile([128, 1], f32, tag=f"sq{hl}")
                    nc.scalar.activation(
                        scrap2[:].rearrange("p (x b) -> p x b", b=16),
                        Os[:, :, hl, :], AF.Square, accum_out=sq1[:],
                    )
                    nc.vector.tensor_scalar_mul(
                        st2[:, 2 * hl:2 * hl + 1], sum1[:], 1.0 / npe
                    )
                    nc.vector.tensor_scalar_mul(
                        st2[:, 2 * hl + 1:2 * hl + 2], sq1[:], 1.0 / npe
                    )
                hs1 = psD.tile([1, 4], f32, tag="psd", name="hs1")
                nc.tensor.matmul(hs1[:], ones128, st2[:])
                hsb = st1.tile([1, 4], f32, tag="hsb")
                nc.vector.tensor_copy(hsb[:], hs1[:])
                # local affine params -> [A_hl0, B_hl0, A_hl1, B_hl1,
                #                         a1'_hl0, a1'_hl1]
                arst = st1.tile([1, 6], f32, tag="arst")
                for hl in range(HL):
                    ms = hsb[:, 2 * hl:2 * hl + 1]       # 32*mean
                    qs = hsb[:, 2 * hl + 1:2 * hl + 2]   # 1024*E[x^2]
                    m2 = st1.tile([1, 1], f32, tag=f"m2_{hl}")
                    nc.vector.tensor_tensor(m2[:], ms, ms, op=OP.mult)
                    v32 = st1.tile([1, 1], f32, tag=f"v32_{hl}")
                    nc.vector.tensor_tensor(v32[:], qs, m2[:],
                                            op=OP.subtract)
                    varp = st1.tile([1, 1], f32, tag=f"vp_{hl}")
                    nc.vector.tensor_scalar(
                        varp[:], v32[:], 1.0 / (SCALE_O * SCALE_O), EPS,
                        op0=OP.mult, op1=OP.add,
                    )
                    rs = st1.tile([1, 1], f32, tag=f"rs_{hl}")
                    rsqrt_dve(st1, rs[:], varp[:], 1, f"g{hl}")
                    a1 = st1.tile([1, 1], f32, tag=f"a1_{hl}")
                    nc.vector.tensor_tensor(
                        a1[:], bnp1_sb[:, 2 * hl:2 * hl + 1], rs[:],
                        op=OP.mult,
                    )
                    inva = st1.tile([1, 1], f32, tag=f"ia_{hl}")
                    nc.vector.reciprocal(inva[:], a1[:])
                    mm = st1.tile([1, 1], f32, tag=f"mm_{hl}")
                    nc.vector.tensor_scalar_mul(mm[:], ms, 1.0 / SCALE_O)
                    am = st1.tile([1, 1], f32, tag=f"am_{hl}")
                    nc.vector.tensor_tensor(am[:], a1[:], mm[:],
                                            op=OP.mult)
                    c1 = st1.tile([1, 1], f32, tag=f"c1_{hl}")
                    nc.vector.tensor_tensor(
                        c1[:], bnp1_sb[:, 2 * hl + 1:2 * hl + 2], am[:],
                        op=OP.subtract,
                    )
                    ci = st1.tile([1, 1], f32, tag=f"ci_{hl}")
                    nc.vector.tensor_tensor(ci[:], c1[:], inva[:],
                                            op=OP.mult)
                    nc.vector.tensor_scalar_mul(
                        arst[:, 2 * hl:2 * hl + 1], ci[:],
                        SCALE_W * SCALE_O,
                    )
                    nc.vector.tensor_scalar_mul(
                        arst[:, 2 * hl + 1:2 * hl + 2], inva[:],
                        SCALE_W * SCALE_O,
                    )
                    nc.vector.tensor_scalar_mul(
                        arst[:, 4 + hl:5 + hl], a1[:], INV_SCALES,
                    )
                arst_d = dp.tile([1, 6], f32, tag="arstd")
                nc.sync.dma_start(arst_d[:], arst[:])
                absh = dp.tile([N_CORES, 1, 6], f32, tag="absh",
                               addr_space="Shared")
                nc.gpsimd.collective_compute(
                    "AllGather", OP.bypass, replica_groups=RG,
                    ins=[arst_d[:].opt()], outs=[absh[:].opt()],
                )
                ab8 = st1.tile([N_CORES, 6], f32, tag="ab8")
                nc.sync.dma_start(ab8[:], absh[:, 0, :])
                # transpose per-head params to rows
                abT = []
                for s in range(2):
                    tp = psD.tile([2, N_CORES], f32, tag="psd",
                                  name=f"abTp{s}")
                    nc.tensor.transpose(tp[:], ab8[:, 2 * s:2 * s + 2],
                                        eye8_sb)
                    sb = st1.tile([2, N_CORES], f32, tag=f"abT{s}")
                    nc.vector.tensor_copy(sb[:], tp[:])
                    abT.append(sb)
                a1s = []
                for hl in range(HL):
                    tp = psD.tile([1, N_CORES], f32, tag="psd",
                                  name=f"a1Tp{hl}")
                    nc.tensor.transpose(tp[:], ab8[:, 4 + hl:5 + hl],
                                        eye8_sb)
                    sb = st1.tile([1, N_CORES], f32, tag=f"a1s{hl}")
                    nc.vector.tensor_copy(sb[:], tp[:])
                    a1s.append(sb)
                # D rows: dts[(j,hl)] = [8c, 128p] = A_hl[c]*w1s + B_hl[c]*b1
                dts = st1.tile([8, 10 * 128], bf16, tag="dts")
                for j in range(5):
                    for hl in range(HL):
                        dtp = psD.tile([8, 128], f32, tag="psd",
                                       name=f"dtp{j}_{hl}")
                        nc.tensor.matmul(
                            dtp[:], abT[hl][:],
                            wb_sb[:, j * 128:(j + 1) * 128],
                        )
                        nc.vector.tensor_copy(
                            dts[:, (j * 2 + hl) * 128:
                                (j * 2 + hl + 1) * 128],
                            dtp[:],
                        )
                # a1 broadcast [64, 16] (col = hl*8 + c)
                a1bc = st1.tile([KN, 16], f32, tag="a1bc")
                for hl in range(HL):
                    a1p = psD.tile([KN, N_CORES], f32, tag="psd",
                                   name=f"a1p{hl}")
                    nc.tensor.matmul(a1p[:], bc1_sb[:, 0:KN], a1s[hl][:])
                    nc.vector.tensor_copy(
                        a1bc[:, hl * 8:(hl + 1) * 8], a1p[:]
                    )
                # D-add into h1 psums
                for j in range(5):
                    for hl in range(HL):
                        nc.tensor.matmul(
                            h1ps[j][:],
                            dts[:, (j * 2 + hl) * 128:
                                (j * 2 + hl + 1) * 128],
                            ind_sb[hl],
                            start=False, stop=(hl == HL - 1),
                            skip_group_check=True,
                        )

                # ---------- leaky + W2 + scale + AllReduce + sigmoid ----
                h1sb = [
                    fp.tile([128, T], bf16, tag=f"h1s_{j}", name=f"h1s{j}")
                    for j in range(5)
                ]
                h1af = [
                    fp.tile([128, T], bf16, tag=f"h1a_{j}", name=f"h1a{j}")
                    for j in range(5)
                ]
                for j in range(5):
                    nc.scalar.activation(
                        h1af[j][:], h1ps[j][:], AF.Identity,
                        bias=0.0, scale=1.0,
                    )
                    nc.vector.scalar_tensor_tensor(
                        h1sb[j][:], h1af[j][:], SLOPE, h1af[j][:],
                        op0=OP.mult, op1=OP.max,
                    )
                ps2 = psH.tile([KN, T], f32, tag="out2")
                for j in range(5):
                    nc.tensor.matmul(
                        ps2[:], w2_sb[j], h1sb[j][:],
                        start=(j == 0), stop=(j == 4),
                    )
                o2f = fp.tile([KN, T], f32, tag="o2f")
                nc.vector.tensor_copy(o2f[:], ps2[:])
                o2s = fp.tile([KN, T], f32, tag="o2s")
                for h in range(NQ):
                    for c in range(N_CORES):
                        for hl in range(HL):
                            g = h * 256 + c * 32 + hl * 16
                            nc.vector.tensor_scalar(
                                o2s[:, g:g + 16], o2f[:, g:g + 16],
                                a1bc[:, hl * 8 + c:hl * 8 + c + 1], None,
                                op0=OP.mult,
                            )
                arin = dp.tile([KN, T], f32, tag="arin")
                nc.sync.dma_start(arin[:], o2s[:])
                arout = dp.tile([KN, T], f32, tag="arout",
                                addr_space="Shared")
                nc.gpsimd.collective_compute(
                    "AllReduce", OP.add, replica_groups=RG,
                    ins=[arin[:].opt()], outs=[arout[:].opt()],
                )
                arsb = fp.tile([KN, T], f32, tag="arsb")
                nc.sync.dma_start(arsb[:], arout[:])
                fin = fp.tile([KN, T], f32, tag="fin")
                nc.scalar.activation(
                    fin[:], arsb[:], AF.Sigmoid, bias=b2_sb, scale=1.0
                )
                nc.sync.dma_start(out_d, fin[:])

    nc.compile()
    return nc


def _dup_wT(W, c):
    W = np.asarray(W, np.float32)
    cols = [W[8 * c + ST * hl: 8 * c + ST * hl + KN, :].T for hl in range(HL)]
    return np.concatenate(cols, axis=1)


def _dup_b(b, c):
    b = np.asarray(b, np.float32)
    rows = [b[8 * c + ST * hl: 8 * c + ST * hl + KN] for hl in range(HL)]
    return np.ascontiguousarray(np.concatenate(rows))


def _prep_in_maps(inputs):
    import ml_dtypes

    f = np.float32
    bf = ml_dtypes.bfloat16
    f8 = ml_dtypes.float8_e4m3
    q = np.asarray(inputs["q"], f)
    k = np.asarray(inputs["k"], f)
    v = np.asarray(inputs["v"], f)
    qh = np.ascontiguousarray(
        q[:, 0].transpose(1, 0, 2).reshape(FN, BS * SL).astype(bf))
    kh = np.ascontiguousarray(
        k[:, 0].transpose(1, 0, 2).reshape(FN, BS * SL).astype(bf))
    vh = np.ascontiguousarray(
        v[:, 0].transpose(1, 0, 2).reshape(FN, BS * SL).astype(bf))
    W1 = np.asarray(inputs["W1"], f)
    W1p = np.zeros((HIDP, SL * KN), f)
    W1p[:HID] = W1
    # device contraction row ((ic*64+kk)*128+p) = orig col ((ic*128+p)*64+kk)
    W1r = W1p.reshape(HIDP, IC, 128, KN).transpose(1, 3, 2, 0).reshape(
        SL * KN, HIDP)
    # DoubleRow pairs: pair axis = ic -> [m=kk, 128 p, 2 (ic), HIDP]
    W1m = W1r.reshape(IC, KN, 128, HIDP).transpose(1, 2, 0, 3)
    w1sum = np.zeros((HIDP,), f)
    w1sum[:HID] = W1.sum(axis=1)
    b1p = np.zeros((HIDP,), f)
    b1p[:HID] = np.asarray(inputs["b1"], f)
    W2T = np.zeros((HIDP, KN), f)
    W2T[:HID] = np.asarray(inputs["W2"], f).T
    b2 = np.asarray(inputs["b2"], f)
    # token order: t = h*256 + c*32 + hl*16 + b16; head = c*2 + hl
    tt = np.arange(T)
    thead = 2 * ((tt % 256) // 32) + (tt % 32) // 16
    in_maps = []
    for c in range(N_CORES):
        h0 = HL * c
        packf = np.zeros((128, PCW), f)
        packf[:, PC_BQ] = _dup_b(inputs["bq"], c)
        packf[:, PC_BK] = _dup_b(inputs["bk"], c)
        packf[:, PC_BV] = _dup_b(inputs["bv"], c)
        for hl in range(HL):
            packf[KN * hl:KN * (hl + 1), PC_MASK + hl] = 1.0 / KN
            packf[hl, PC_SEL + hl * KN:PC_SEL + (hl + 1) * KN] = 1.0
        packf[0:KN, PC_B2] = b2
        packf[:, PC_ONES] = 1.0 / 128.0
        packf[0, PC_BC1:PC_BC1 + 128] = 1.0
        for hl in range(HL):
            packf[hl, PC_BNP:PC_BNP + 8] = [
                inputs["gq"][h0 + hl], inputs["beq"][h0 + hl],
                inputs["gk"][h0 + hl], inputs["bek"][h0 + hl],
                inputs["gv"][h0 + hl], inputs["bev"][h0 + hl],
                inputs["g1"][h0 + hl], inputs["be1"][h0 + hl],
            ]
        packf[0, PC_BNP1:PC_BNP1 + 4] = [
            inputs["g1"][h0], inputs["be1"][h0],
            inputs["g1"][h0 + 1], inputs["be1"][h0 + 1],
        ]
        packf[0:8, PC_EYE8:PC_EYE8 + 8] = np.eye(8, dtype=f)
        packf[0:2, PC_MAGIC] = np.frombuffer(
            np.array([0x5F3759DF, 0x5F3759DF], np.uint32).tobytes(),
            dtype=f)
        packf[0, PC_WB:PC_WB + 5 * 128] = w1sum[c * HSH:(c + 1) * HSH]
        packf[1, PC_WB:PC_WB + 5 * 128] = b1p[c * HSH:(c + 1) * HSH]
        packb = np.zeros((128, PBW), f)
        packb[:, 0:128] = np.eye(128, dtype=f)
        W2c = W2T[c * HSH:(c + 1) * HSH, :]
        for j in range(5):
            packb[:, PB_W2 + j * KN:PB_W2 + (j + 1) * KN] = \
                W2c[j * 128:(j + 1) * 128, :]
        for hl in range(HL):
            for cc in range(N_CORES):
                head = cc * 2 + hl
                packb[cc, PB_IND + hl * T:PB_IND + (hl + 1) * T] = \
                    (thead == head).astype(f)
        w1c = np.ascontiguousarray(
            (W1m[:, :, :, c * HSH:(c + 1) * HSH] * SCALE_W)
            .transpose(0, 1, 2, 3)
            .reshape(NM, 128, 2 * HSH).astype(f8))
        m = {
            "qh": qh, "kh": kh, "vh": vh,
            "wqT": np.ascontiguousarray(_dup_wT(inputs["Wq"], c).astype(bf)),
            "wkT": np.ascontiguousarray(_dup_wT(inputs["Wk"], c).astype(bf)),
            "wvT": np.ascontiguousarray(_dup_wT(inputs["Wv"], c).astype(bf)),
            "packf": packf,
            "packb": np.ascontiguousarray(packb.astype(bf)),
            "w1a": np.ascontiguousarray(w1c[:NMA]),
            "w1b": np.ascontiguousarray(w1c[NMA:]),
        }
        in_maps.append(m)
    return in_maps


def _unshard(o):
    # out cols: t = h*256 + c*32 + hl*16 + b16; head = c*2+hl, b = h*16+b16
    out = (
        np.asarray(o, np.float32)
        .reshape(KN, NQ, N_CORES, HL, QB)
        .transpose(1, 4, 2, 3, 0)
        .reshape(BS, HEADS, KN)[:, None]
    )
    return np.ascontiguousarray(out.astype(np.float32))


def kernel(**inputs):
    global _prog
    if _prog is None:
        _prog = _build()
    from concourse.bass_utils import run_bass_kernel_spmd

    in_maps = _prep_in_maps(inputs)
    res = run_bass_kernel_spmd(_prog, in_maps, list(range(N_CORES)))
    return _unshard(res.results[0]["out"])
